# revision 1
# baseline (speedup 1.0000x reference)
"""Trainium2 Bass kernel for the DeepFace-style CNN (nn_DeepFace_10574209482846).

Sharding: pure data parallel — batch 2048 split as 256 images per core
across 8 cores; all weights replicated (host-preprocessed into matmul-
friendly block-diagonal / stacked layouts, cast to bf16).

Per-core layout: the 256 images form 4 "groups" of 64. Dense convs run
with channels on partitions and (b, y, x) on the free dim, 4 groups (or
2-group pairs) stacked on partitions via block-diagonal weights so the
128-wide PE array is filled.  Locally-connected layers use a
batch-contiguous (y, x, b) layout with a +1-column-shifted replica on
partitions 64..127 so two kernel taps contract per matmul (K=128).
"""

import numpy as np
import concourse.bass as bass
import concourse.bacc as bacc
import concourse.tile as tile
import concourse.mybir as mybir
from concourse import bass_utils

bf16 = mybir.dt.bfloat16
f32 = mybir.dt.float32
BF = mybir.dt.np(bf16)  # ml_dtypes.bfloat16

N_CORES = 8
B_FULL = 2048
B_CORE = 256          # images per core
SB = 8                # images per sub-batch (2 per group)
NSB = B_CORE // SB    # 32
BSB = SB // 4         # 2 images per group per sub-batch

TAPS3 = [(di, dj) for di in range(3) for dj in range(3)]

_CACHE = {}


def _build_module(nsb=NSB, phase2=True):
    nc = bacc.Bacc("TRN2", target_bir_lowering=False, debug=False,
                   enable_asserts=True, num_devices=N_CORES)

    # ---- DRAM I/O ----
    x_d = nc.dram_tensor("x", [B_CORE, 5, 3600], bf16, kind="ExternalInput").ap()
    w1bd_d = nc.dram_tensor("w1bd", [20, 9 * 128], bf16, kind="ExternalInput").ap()
    b1t_d = nc.dram_tensor("b1t", [128, 1], f32, kind="ExternalInput").ap()
    w2abd_d = nc.dram_tensor("w2abd", [128, 9 * 128], bf16, kind="ExternalInput").ap()
    b2at_d = nc.dram_tensor("b2at", [128, 1], f32, kind="ExternalInput").ap()
    w2bbd_d = nc.dram_tensor("w2bbd", [128, 9 * 128], bf16, kind="ExternalInput").ap()
    b2bt_d = nc.dram_tensor("b2bt", [128, 1], f32, kind="ExternalInput").ap()
    lw3p_d = nc.dram_tensor("lw3p", [81, 128, 640], bf16, kind="ExternalInput").ap()
    lw3s_d = nc.dram_tensor("lw3s", [81, 64, 320], bf16, kind="ExternalInput").ap()
    lb3_d = nc.dram_tensor("lb3t", [64, 81], f32, kind="ExternalInput").ap()
    lw4p_d = nc.dram_tensor("lw4p", [25, 128, 640], bf16, kind="ExternalInput").ap()
    lw4s_d = nc.dram_tensor("lw4s", [25, 64, 320], bf16, kind="ExternalInput").ap()
    lb4_d = nc.dram_tensor("lb4t", [64, 25], f32, kind="ExternalInput").ap()
    lw5p_d = nc.dram_tensor("lw5p", [9, 128, 192], bf16, kind="ExternalInput").ap()
    lw5s_d = nc.dram_tensor("lw5s", [9, 64, 192], bf16, kind="ExternalInput").ap()
    lb5_d = nc.dram_tensor("lb5t", [64, 9], f32, kind="ExternalInput").ap()
    hwch_d = nc.dram_tensor("hwch", [64, 18], bf16, kind="ExternalInput").ap()
    logits_d = nc.dram_tensor("logits", [2, B_CORE], f32, kind="ExternalOutput").ap()

    Tanh = mybir.ActivationFunctionType.Tanh

    with tile.TileContext(nc) as tc:
        with (
            tc.tile_pool(name="wp", bufs=1) as wp,
            tc.tile_pool(name="lwp", bufs=3) as lwp,
            tc.tile_pool(name="xp", bufs=1) as xp,
            tc.tile_pool(name="h1p", bufs=1) as h1p,
            tc.tile_pool(name="h2ap", bufs=1) as h2ap,
            tc.tile_pool(name="bigp", bufs=1) as bigp,
            tc.tile_pool(name="cps", bufs=4, space="PSUM") as cps,
            tc.tile_pool(name="lps", bufs=3, space="PSUM") as lps,
            tc.tile_pool(name="hps", bufs=1, space="PSUM") as hps,
        ):
            # ---- persistent weights ----
            w1bd = wp.tile([20, 9 * 128], bf16)
            nc.sync.dma_start(w1bd[:], w1bd_d[:])
            b1t = wp.tile([128, 1], f32)
            nc.sync.dma_start(b1t[:], b1t_d[:])
            w2abd = wp.tile([128, 9 * 128], bf16)
            nc.sync.dma_start(w2abd[:], w2abd_d[:])
            b2at = wp.tile([128, 1], f32)
            nc.sync.dma_start(b2at[:], b2at_d[:])
            w2bbd = wp.tile([128, 9 * 128], bf16)
            nc.sync.dma_start(w2bbd[:], w2bbd_d[:])
            b2bt = wp.tile([128, 1], f32)
            nc.sync.dma_start(b2bt[:], b2bt_d[:])
            lb3t = wp.tile([64, 81], f32)
            nc.sync.dma_start(lb3t[:], lb3_d[:])
            lb4t = wp.tile([64, 25], f32)
            nc.sync.dma_start(lb4t[:], lb4_d[:])
            lb5t = wp.tile([64, 9], f32)
            nc.sync.dma_start(lb5t[:], lb5_d[:])
            hwch = wp.tile([64, 18], bf16)
            nc.sync.dma_start(hwch[:], hwch_d[:])

            # ---- persistent activations (batch-contiguous, (y, x, b)) ----
            h2brep = bigp.tile([128, 169 * 256], bf16)   # rows 0-63 h2b, 64-127 +1col
            h3rep = bigp.tile([128, 81 * 256], bf16)
            h4rep = bigp.tile([128, 25 * 256], bf16)
            h5t = bigp.tile([64, 9 * 256], bf16)

            h2bv = h2brep[:].rearrange("c (y x b) -> c b y x", y=13, x=13, b=256)

            # ================= phase 1: conv1 -> conv2a -> conv2b =============
            for sb in range(nsb):
                # load x sub-batch: 4 groups stacked at partition rows 5g
                x_t = xp.tile([20, BSB * 3600], bf16, tag="x")
                for g in range(4):
                    b0 = 64 * g + BSB * sb
                    src = x_d[b0:b0 + BSB, :, :].rearrange("b c m -> c b m")
                    nc.sync.dma_start(
                        x_t[5 * g:5 * g + 5, :].rearrange("c (b m) -> c b m", b=BSB),
                        src)
                xv = x_t[:].rearrange("c (b h w) -> c b h w", b=BSB, h=60, w=60)

                # ---- conv1: K=20 block-diag over 4 groups, M=128 = 4x32co ----
                h1_t = h1p.tile([128, BSB * 841], bf16, tag="h1")
                h1v = h1_t[:].rearrange("c (b h w) -> c b h w", b=BSB, h=29, w=29)
                for (y0, ny) in [(0, 8), (8, 8), (16, 8), (24, 5)]:
                    ps = cps.tile([128, BSB * 8 * 29], f32, tag="cps")
                    psw = ps[:, :BSB * ny * 29]
                    for t, (di, dj) in enumerate(TAPS3):
                        rhs = xv[:, :, 2 * y0 + di: 2 * y0 + di + 2 * ny - 1: 2,
                                 dj: dj + 57: 2]
                        nc.tensor.matmul(psw, w1bd[:, 128 * t:128 * (t + 1)], rhs,
                                         start=(t == 0), stop=(t == 8))
                    nc.scalar.activation(h1v[:, :, y0:y0 + ny, :], psw, Tanh,
                                         bias=b1t[:])

                # ---- conv2a: 2 pairs x (K=64 block-diag), M=128 = 2x64co ----
                h2a_t = {}
                for r in range(2):  # pair r: groups (2r, 2r+1), lhsT rows 64r..
                    h2a_t[r] = h2ap.tile([128, BSB * 729], bf16, tag=f"h2a{r}", name=f"h2a{r}")
                    h2av = h2a_t[r][:].rearrange("c (b h w) -> c b h w",
                                                 b=BSB, h=27, w=27)
                    for (y0, ny) in [(0, 9), (9, 9), (18, 9)]:
                        ps = cps.tile([128, BSB * 9 * 27], f32, tag="cps")
                        psw = ps[:, :BSB * ny * 27]
                        for t, (di, dj) in enumerate(TAPS3):
                            rhs = h1v[64 * r:64 * (r + 1), :,
                                      y0 + di: y0 + di + ny, dj: dj + 27]
                            nc.tensor.matmul(
                                psw,
                                w2abd[64 * r:64 * (r + 1), 128 * t:128 * (t + 1)],
                                rhs, start=(t == 0), stop=(t == 8))
                        nc.scalar.activation(h2av[:, :, y0:y0 + ny, :], psw, Tanh,
                                             bias=b2at[:])

                # ---- conv2b: per pair, K=128 block-diag, stride 2 ----
                for r in range(2):
                    h2av = h2a_t[r][:].rearrange("c (b h w) -> c b h w",
                                                 b=BSB, h=27, w=27)
                    ps = cps.tile([128, BSB * 169], f32, tag="cps")
                    for t, (di, dj) in enumerate(TAPS3):
                        rhs = h2av[:, :, di: di + 25: 2, dj: dj + 25: 2]
                        nc.tensor.matmul(ps[:], w2bbd[:, 128 * t:128 * (t + 1)],
                                         rhs, start=(t == 0), stop=(t == 8))
                    # evacuate: psum rows (g-even 0:64 / g-odd 64:128) into
                    # h2brep[0:64, (y, x, b)] at the right global-b columns
                    psv = ps[:].rearrange("c (b y x) -> c b y x", b=BSB, y=13, x=13)
                    for g2 in range(2):
                        gb = 64 * (2 * r + g2) + BSB * sb
                        nc.scalar.activation(
                            h2bv[0:64, gb:gb + BSB, :, :],
                            psv[64 * g2:64 * (g2 + 1), :, :, :], Tanh,
                            bias=b2bt[64 * g2:64 * (g2 + 1)])

            # replica rows 64-127 = +1 x-column shift (= +256 elements)
            nc.vector.tensor_copy(h2brep[64:128, 0:169 * 256 - 256],
                                  h2brep[0:64, 256:169 * 256])

            if not phase2:
                lg = wp.tile([2, 256], f32, name="lg")
                nc.vector.tensor_copy(lg[:], h2brep[0:2, 0:512].bitcast(f32))
                nc.sync.dma_start(logits_d[:], lg[:])
            if phase2:
                # ================= phase 2: locally-connected stack ===============
                # ---- lconv3: 13x13 -> 9x9, 5x5 taps ----
                for p in range(81):
                    i, j = p // 9, p % 9
                    lwt = lwp.tile([128, 640], bf16, tag="lw3p")
                    nc.sync.dma_start(lwt[:], lw3p_d[p])
                    lws = lwp.tile([64, 320], bf16, tag="lw3s")
                    nc.sync.dma_start(lws[:], lw3s_d[p])
                    ps = lps.tile([64, 256], f32, tag="lps")
                    for q in range(10):        # (u, v0/v0+1) pairs, K=128
                        u, v0 = q // 2, 2 * (q % 2)
                        col = ((i + u) * 13 + (j + v0)) * 256
                        nc.tensor.matmul(ps[:], lwt[:, 64 * q:64 * (q + 1)],
                                         h2brep[:, col:col + 256],
                                         start=(q == 0), stop=False)
                    for s in range(5):         # (u, v=4) singles, K=64
                        col = ((i + s) * 13 + (j + 4)) * 256
                        nc.tensor.matmul(ps[:], lws[:, 64 * s:64 * (s + 1)],
                                         h2brep[0:64, col:col + 256],
                                         start=False, stop=(s == 4))
                    nc.scalar.activation(h3rep[0:64, 256 * p:256 * (p + 1)], ps[:],
                                         Tanh, bias=lb3t[:, p:p + 1])
                nc.vector.tensor_copy(h3rep[64:128, 0:81 * 256 - 256],
                                      h3rep[0:64, 256:81 * 256])

                # ---- lconv4: 9x9 -> 5x5, 5x5 taps ----
                for p in range(25):
                    i, j = p // 5, p % 5
                    lwt = lwp.tile([128, 640], bf16, tag="lw4p")
                    nc.sync.dma_start(lwt[:], lw4p_d[p])
                    lws = lwp.tile([64, 320], bf16, tag="lw4s")
                    nc.sync.dma_start(lws[:], lw4s_d[p])
                    ps = lps.tile([64, 256], f32, tag="lps")
                    for q in range(10):
                        u, v0 = q // 2, 2 * (q % 2)
                        col = ((i + u) * 9 + (j + v0)) * 256
                        nc.tensor.matmul(ps[:], lwt[:, 64 * q:64 * (q + 1)],
                                         h3rep[:, col:col + 256],
                                         start=(q == 0), stop=False)
                    for s in range(5):
                        col = ((i + s) * 9 + (j + 4)) * 256
                        nc.tensor.matmul(ps[:], lws[:, 64 * s:64 * (s + 1)],
                                         h3rep[0:64, col:col + 256],
                                         start=False, stop=(s == 4))
                    nc.scalar.activation(h4rep[0:64, 256 * p:256 * (p + 1)], ps[:],
                                         Tanh, bias=lb4t[:, p:p + 1])
                nc.vector.tensor_copy(h4rep[64:128, 0:25 * 256 - 256],
                                      h4rep[0:64, 256:25 * 256])

                # ---- lconv5: 5x5 -> 3x3, 3x3 taps ----
                for p in range(9):
                    i, j = p // 3, p % 3
                    lwt = lwp.tile([128, 192], bf16, tag="lw5p")
                    nc.sync.dma_start(lwt[:], lw5p_d[p])
                    lws = lwp.tile([64, 192], bf16, tag="lw5s")
                    nc.sync.dma_start(lws[:], lw5s_d[p])
                    ps = lps.tile([64, 256], f32, tag="lps")
                    for q in range(3):         # (u, v=0/1) pairs
                        col = ((i + q) * 5 + (j + 0)) * 256
                        nc.tensor.matmul(ps[:], lwt[:, 64 * q:64 * (q + 1)],
                                         h4rep[:, col:col + 256],
                                         start=(q == 0), stop=False)
                    for s in range(3):         # (u, v=2) singles
                        col = ((i + s) * 5 + (j + 2)) * 256
                        nc.tensor.matmul(ps[:], lws[:, 64 * s:64 * (s + 1)],
                                         h4rep[0:64, col:col + 256],
                                         start=False, stop=(s == 2))
                    nc.scalar.activation(h5t[:, 256 * p:256 * (p + 1)], ps[:],
                                         Tanh, bias=lb5t[:, p:p + 1])

                # ---- head: logits[o, b] = sum_f hw[o, f] feat[f, b] (h5 part) ----
                psh = hps.tile([2, 256], f32)
                for yx in range(9):
                    nc.tensor.matmul(psh[:], hwch[:, 2 * yx:2 * yx + 2],
                                     h5t[:, 256 * yx:256 * (yx + 1)],
                                     start=(yx == 0), stop=(yx == 8))
                lg = wp.tile([2, 256], f32)
                nc.vector.tensor_copy(lg[:], psh[:])
                nc.sync.dma_start(logits_d[:], lg[:])

    nc.compile()
    return nc


def _prep_weights(w1, b1, w2a, b2a, w2b, b2b, lw3, lb3, lw4, lb4, lw5, lb5, hw):
    """Host-side reshape of weights into the on-chip matmul layouts."""
    out = {}
    w1bd = np.zeros((20, 9, 128), np.float32)
    for t, (di, dj) in enumerate(TAPS3):
        blk = w1[:, :, di, dj].T          # [5ci, 32co]
        for g in range(4):
            w1bd[5 * g:5 * g + 5, t, 32 * g:32 * g + 32] = blk
    out["w1bd"] = w1bd.reshape(20, 9 * 128).astype(BF)
    out["b1t"] = np.tile(b1, 4)[:, None].astype(np.float32)

    w2abd = np.zeros((128, 9, 128), np.float32)
    for t, (di, dj) in enumerate(TAPS3):
        blk = w2a[:, :, di, dj].T         # [32ci, 64co]
        for r in range(2):                # strip copy for pair-B at rows 64+
            for g2 in range(2):
                w2abd[64 * r + 32 * g2:64 * r + 32 * (g2 + 1), t,
                      64 * g2:64 * (g2 + 1)] = blk
    out["w2abd"] = w2abd.reshape(128, 9 * 128).astype(BF)
    out["b2at"] = np.tile(b2a, 2)[:, None].astype(np.float32)

    w2bbd = np.zeros((128, 9, 128), np.float32)
    for t, (di, dj) in enumerate(TAPS3):
        blk = w2b[:, :, di, dj].T         # [64ci, 64co]
        for g2 in range(2):
            w2bbd[64 * g2:64 * (g2 + 1), t, 64 * g2:64 * (g2 + 1)] = blk
    out["w2bbd"] = w2bbd.reshape(128, 9 * 128).astype(BF)
    out["b2bt"] = np.tile(b2b, 2)[:, None].astype(np.float32)

    def lc_pack(lw, Ho, Wo, kh, kw):
        npos = Ho * Wo
        npair = kh * (kw // 2)
        nsing = kh
        lp = np.zeros((npos, 128, npair * 64), np.float32)
        ls = np.zeros((npos, 64, nsing * 64), np.float32)
        for p in range(npos):
            i, j = p // Wo, p % Wo
            for q in range(npair):
                u, v0 = q // (kw // 2), 2 * (q % (kw // 2))
                lp[p, 0:64, 64 * q:64 * (q + 1)] = lw[i, j, :, :, u, v0].T
                lp[p, 64:128, 64 * q:64 * (q + 1)] = lw[i, j, :, :, u, v0 + 1].T
            for s in range(nsing):
                ls[p, 0:64, 64 * s:64 * (s + 1)] = lw[i, j, :, :, s, kw - 1].T
        return lp.astype(BF), ls.astype(BF)

    out["lw3p"], out["lw3s"] = lc_pack(lw3, 9, 9, 5, 5)
    out["lb3t"] = np.ascontiguousarray(
        lb3.transpose(2, 0, 1).reshape(64, 81)).astype(np.float32)
    out["lw4p"], out["lw4s"] = lc_pack(lw4, 5, 5, 5, 5)
    out["lb4t"] = np.ascontiguousarray(
        lb4.transpose(2, 0, 1).reshape(64, 25)).astype(np.float32)
    out["lw5p"], out["lw5s"] = lc_pack(lw5, 3, 3, 3, 3)
    out["lb5t"] = np.ascontiguousarray(
        lb5.transpose(2, 0, 1).reshape(64, 9)).astype(np.float32)

    # head: feature f = co*9 + yx; chunk yx -> [64co, 2]
    out["hwch"] = np.ascontiguousarray(
        hw[:, :576].reshape(2, 64, 9).transpose(1, 2, 0).reshape(64, 18)
    ).astype(BF)
    return out


def kernel(x, info, w1, b1, w2a, b2a, w2b, b2b, lw3, lb3, lw4, lb4, lw5, lb5,
           hw, hb, _trace=False):
    x = np.asarray(x, np.float32)
    if "nc" not in _CACHE:
        _CACHE["nc"] = _build_module()
    nc = _CACHE["nc"]

    wts = _prep_weights(np.asarray(w1, np.float32), np.asarray(b1, np.float32),
                        np.asarray(w2a, np.float32), np.asarray(b2a, np.float32),
                        np.asarray(w2b, np.float32), np.asarray(b2b, np.float32),
                        np.asarray(lw3, np.float32), np.asarray(lb3, np.float32),
                        np.asarray(lw4, np.float32), np.asarray(lb4, np.float32),
                        np.asarray(lw5, np.float32), np.asarray(lb5, np.float32),
                        np.asarray(hw, np.float32))

    xb = np.ascontiguousarray(x.reshape(B_FULL, 5, 3600)).astype(BF)
    in_maps = []
    for c in range(N_CORES):
        m = dict(wts)
        m["x"] = xb[c * B_CORE:(c + 1) * B_CORE]
        in_maps.append(m)

    res = bass_utils.run_bass_kernel_spmd(
        nc, in_maps, core_ids=list(range(N_CORES)), trace=_trace)
    if _trace:
        _CACHE["last_results"] = res

    logits = np.concatenate(
        [res.results[c]["logits"].T for c in range(N_CORES)], axis=0)  # [2048, 2]

    # host-side tail: info contribution + bias + softmax (fp32)
    info = np.asarray(info, np.float32)
    hw = np.asarray(hw, np.float32)
    hb = np.asarray(hb, np.float32)
    logits = logits + info @ hw[:, 576:].T + hb[None, :]
    m = logits.max(axis=1, keepdims=True)
    e = np.exp(logits - m)
    return (e / e.sum(axis=1, keepdims=True)).astype(np.float32)



# revision 7
# speedup vs baseline: 1.1491x; 1.1491x over previous
"""Trainium2 Bass kernel for the DeepFace-style CNN (nn_DeepFace_10574209482846).

Sharding: pure data parallel — batch 2048 split as 256 images per core
across 8 cores; all weights replicated (host-preprocessed into matmul-
friendly layouts, cast to bf16).

Per-core layout: 256 images = 4 groups of 64; sub-batches of 8 images
(2 per group).  Dense convs contract the 3 row-taps (di) inside K via
physically row-shifted input stacks (built by DMA for conv1, by cheap
contiguous DVE copies for conv2a); the 3 column-taps (dj) are free-dim
offsets.  conv2b keeps the 2-group block-diagonal K=128 form (already
optimal).  Locally-connected layers use a batch-contiguous (y, x, b)
layout with a +1-column-shifted replica on partitions 64..127 (written
directly by dual scalar activations) so two kernel taps contract per
matmul (K=128).
"""

import numpy as np
import concourse.bass as bass
import concourse.bacc as bacc
import concourse.tile as tile
import concourse.mybir as mybir
from concourse import bass_utils

bf16 = mybir.dt.bfloat16
f32 = mybir.dt.float32
BF = mybir.dt.np(bf16)  # ml_dtypes.bfloat16

N_CORES = 8
B_FULL = 2048
B_CORE = 256          # images per core
SB = 8                # images per sub-batch (2 per group)
NSB = B_CORE // SB    # 32
BSB = SB // 4         # 2 images per group per sub-batch

TAPS3 = [(di, dj) for di in range(3) for dj in range(3)]

_CACHE = {}


def _build_module(nsb=NSB, phase2=True):
    nc = bacc.Bacc("TRN2", target_bir_lowering=False, debug=False,
                   enable_asserts=True, num_devices=N_CORES)

    # ---- DRAM I/O ----
    # x pre-split on host into row-parity planes: plane 0 = even rows
    # 0,2,..,58 (30), plane 1 = odd rows 1,3,..,59 (30).
    x_d = nc.dram_tensor("x", [B_CORE, 2, 5, 1800], bf16,
                         kind="ExternalInput").ap()
    w1di_d = nc.dram_tensor("w1di", [60, 3 * 128], bf16, kind="ExternalInput").ap()
    b1t_d = nc.dram_tensor("b1t", [128, 1], f32, kind="ExternalInput").ap()
    w2adi_d = nc.dram_tensor("w2adi", [96, 3 * 64], bf16, kind="ExternalInput").ap()
    b2at_d = nc.dram_tensor("b2at", [64, 1], f32, kind="ExternalInput").ap()
    w2bbd_d = nc.dram_tensor("w2bbd", [128, 9 * 128], bf16, kind="ExternalInput").ap()
    b2bt_d = nc.dram_tensor("b2bt", [128, 1], f32, kind="ExternalInput").ap()
    lw3p_d = nc.dram_tensor("lw3p", [81, 128, 640], bf16, kind="ExternalInput").ap()
    lw3s_d = nc.dram_tensor("lw3s", [81, 64, 320], bf16, kind="ExternalInput").ap()
    lb3_d = nc.dram_tensor("lb3t", [64, 81], f32, kind="ExternalInput").ap()
    lw4p_d = nc.dram_tensor("lw4p", [25, 128, 640], bf16, kind="ExternalInput").ap()
    lw4s_d = nc.dram_tensor("lw4s", [25, 64, 320], bf16, kind="ExternalInput").ap()
    lb4_d = nc.dram_tensor("lb4t", [64, 25], f32, kind="ExternalInput").ap()
    lw5p_d = nc.dram_tensor("lw5p", [9, 128, 192], bf16, kind="ExternalInput").ap()
    lw5s_d = nc.dram_tensor("lw5s", [9, 64, 192], bf16, kind="ExternalInput").ap()
    lb5_d = nc.dram_tensor("lb5t", [64, 9], f32, kind="ExternalInput").ap()
    hwch_d = nc.dram_tensor("hwch", [64, 18], bf16, kind="ExternalInput").ap()
    logits_d = nc.dram_tensor("logits", [2, B_CORE], f32, kind="ExternalOutput").ap()

    Tanh = mybir.ActivationFunctionType.Tanh

    with tile.TileContext(nc) as tc:
        with (
            tc.tile_pool(name="wp", bufs=1) as wp,
            tc.tile_pool(name="lwp", bufs=2) as lwp,
            tc.tile_pool(name="xp", bufs=2) as xp,
            tc.tile_pool(name="h1p", bufs=1) as h1p,
            tc.tile_pool(name="stkp", bufs=2) as stkp,
            tc.tile_pool(name="h2ap", bufs=1) as h2ap,
            tc.tile_pool(name="bigp", bufs=1) as bigp,
            tc.tile_pool(name="cps", bufs=4, space="PSUM") as cps,
            tc.tile_pool(name="lps", bufs=3, space="PSUM") as lps,
            tc.tile_pool(name="hps", bufs=1, space="PSUM") as hps,
        ):
            # ---- persistent weights ----
            w1di = wp.tile([60, 3 * 128], bf16)
            nc.sync.dma_start(w1di[:], w1di_d[:])
            b1t = wp.tile([128, 1], f32)
            nc.sync.dma_start(b1t[:], b1t_d[:])
            w2adi = wp.tile([96, 3 * 64], bf16)
            nc.sync.dma_start(w2adi[:], w2adi_d[:])
            b2at = wp.tile([64, 1], f32)
            nc.sync.dma_start(b2at[:], b2at_d[:])
            w2bbd = wp.tile([128, 9 * 128], bf16)
            nc.sync.dma_start(w2bbd[:], w2bbd_d[:])
            b2bt = wp.tile([128, 1], f32)
            nc.sync.dma_start(b2bt[:], b2bt_d[:])
            lb3t = wp.tile([64, 81], f32)
            nc.sync.dma_start(lb3t[:], lb3_d[:])
            lb4t = wp.tile([64, 25], f32)
            nc.sync.dma_start(lb4t[:], lb4_d[:])
            lb5t = wp.tile([64, 9], f32)
            nc.sync.dma_start(lb5t[:], lb5_d[:])
            hwch = wp.tile([64, 18], bf16)
            nc.sync.dma_start(hwch[:], hwch_d[:])

            # ---- persistent activations (batch-contiguous, (y, x, b)) ----
            h2brep = bigp.tile([128, 169 * 256], bf16)   # rows 0-63 h2b, 64-127 +1col
            h3rep = bigp.tile([128, 81 * 256], bf16)
            h4rep = bigp.tile([128, 25 * 256], bf16)
            h5t = bigp.tile([64, 9 * 256], bf16)

            h2bv = h2brep[:].rearrange("c (y x b) -> c b y x", y=13, x=13, b=256)

            # ================= phase 1: conv1 -> conv2a -> conv2b =============
            # software-pipelined: iter sb runs conv1(sb)+copies(sb), then
            # conv2a(sb-1)+conv2b(sb-1), so DVE copies hide under PE work.
            C1STRIPS = [(0, 8), (8, 8), (16, 8), (24, 5)]
            C2STRIPS = [(0, 9), (9, 9), (18, 9)]

            def load_x(sb):
                # row-shifted di-stack: partition (g, di, c) = 15g+5di+c,
                # free (b, y, x) with tile row y = input row 2y+di.
                xs = xp.tile([60, BSB * 29 * 60], bf16, tag="xs", name="xs")
                xsb = xs[:].rearrange("p (b m) -> p b m", b=BSB)
                for g in range(4):
                    b0 = 64 * g + BSB * sb
                    base = x_d[b0:b0 + BSB]          # [BSB, 2, 5, 1800]
                    for di, (pl, r0) in enumerate([(0, 0), (1, 0), (0, 1)]):
                        src = base[:, pl, :, 60 * r0:60 * r0 + 1740].rearrange(
                            "b c m -> c b m")
                        nc.sync.dma_start(
                            xsb[15 * g + 5 * di:15 * g + 5 * di + 5, :, :], src)
                return xs

            def conv1(xs):
                xsv = xs[:].rearrange("p (b y x) -> p b y x", b=BSB, y=29, x=60)
                h1 = h1p.tile([128, BSB * 841], bf16, tag="h1", name="h1")
                h1v = h1[:].rearrange("c (b h w) -> c b h w", b=BSB, h=29, w=29)
                for (y0, ny) in C1STRIPS:
                    ps = cps.tile([128, 486], f32, tag="cps", name="c1ps")
                    psw = ps[:, :BSB * ny * 29]
                    for dj in range(3):
                        rhs = xsv[:, :, y0:y0 + ny, dj:dj + 57:2]
                        nc.tensor.matmul(psw, w1di[:, 128 * dj:128 * (dj + 1)],
                                         rhs, start=(dj == 0), stop=(dj == 2))
                    nc.scalar.activation(h1v[:, :, y0:y0 + ny, :], psw, Tanh,
                                         bias=b1t[:])
                # di-stack for conv2a: per group [96 = 3di x 32c, (b y x)],
                # di shift = +29 elements (one y row), flat contiguous copy.
                stk = stkp.tile([96, 4 * 1682], bf16, tag="stk", name="stk")
                for g in range(4):
                    for di in range(3):
                        L = 1682 - 29 * di
                        nc.vector.tensor_copy(
                            stk[32 * di:32 * di + 32, 1682 * g:1682 * g + L],
                            h1[32 * g:32 * g + 32, 29 * di:29 * di + L])
                return stk

            def conv23(stk, sb):
                # ---- conv2a: per group, K=96 (3 di x 32 ci), dj by offset ----
                h2a_t = {}
                for r in range(2):
                    h2a_t[r] = h2ap.tile([128, BSB * 729], bf16, tag=f"h2a{r}",
                                         name=f"h2a{r}")
                for g in range(4):
                    r, half = g // 2, g % 2
                    sv = stk[:, 1682 * g:1682 * (g + 1)].rearrange(
                        "p (b h w) -> p b h w", b=BSB, h=29, w=29)
                    h2av = h2a_t[r][:].rearrange("c (b h w) -> c b h w",
                                                 b=BSB, h=27, w=27)
                    for (y0, ny) in C2STRIPS:
                        ps = cps.tile([128, 486], f32, tag="cps", name="c2aps")
                        psw = ps[0:64, :BSB * ny * 27]
                        for dj in range(3):
                            rhs = sv[:, :, y0:y0 + ny, dj:dj + 27]
                            nc.tensor.matmul(
                                psw, w2adi[:, 64 * dj:64 * (dj + 1)], rhs,
                                start=(dj == 0), stop=(dj == 2))
                        nc.scalar.activation(
                            h2av[64 * half:64 * half + 64, :, y0:y0 + ny, :],
                            psw, Tanh, bias=b2at[:])

                # ---- conv2b: per pair, K=128 block-diag, stride 2 ----
                for r in range(2):
                    h2av = h2a_t[r][:].rearrange("c (b h w) -> c b h w",
                                                 b=BSB, h=27, w=27)
                    ps = cps.tile([128, 486], f32, tag="cps", name="c2bps")
                    psw = ps[:, :BSB * 169]
                    for t, (di, dj) in enumerate(TAPS3):
                        rhs = h2av[:, :, di: di + 25: 2, dj: dj + 25: 2]
                        nc.tensor.matmul(psw, w2bbd[:, 128 * t:128 * (t + 1)],
                                         rhs, start=(t == 0), stop=(t == 8))
                    # evacuate: psum rows (g-even 0:64 / g-odd 64:128) into
                    # h2brep[0:64, (y, x, b)] plus the +1-col replica rows.
                    psv = psw.rearrange("c (b y x) -> c b y x", b=BSB, y=13, x=13)
                    psf = psw.rearrange("c (b yx) -> c b yx", b=BSB, yx=169)
                    for g2 in range(2):
                        gb = 64 * (2 * r + g2) + BSB * sb
                        nc.scalar.activation(
                            h2bv[0:64, gb:gb + BSB, :, :],
                            psv[64 * g2:64 * (g2 + 1), :, :, :], Tanh,
                            bias=b2bt[64 * g2:64 * (g2 + 1)])
                        rep = h2brep[64:128, gb:gb + 168 * 256].rearrange(
                            "c (yx b) -> c b yx", b=256)[:, 0:BSB, :]
                        nc.scalar.activation(
                            rep, psf[64 * g2:64 * (g2 + 1), :, 1:169], Tanh,
                            bias=b2bt[64 * g2:64 * (g2 + 1)])

            xs_cur = load_x(0)
            stk_prev = None
            for sb in range(nsb):
                stk_cur = conv1(xs_cur)
                if sb + 1 < nsb:
                    xs_cur = load_x(sb + 1)
                if stk_prev is not None:
                    conv23(stk_prev, sb - 1)
                stk_prev = stk_cur
            conv23(stk_prev, nsb - 1)

            if not phase2:
                lg = wp.tile([2, 256], f32, name="lg")
                nc.vector.tensor_copy(lg[:], h2brep[0:2, 0:512].bitcast(f32))
                nc.sync.dma_start(logits_d[:], lg[:])
            if phase2:
                # ================= phase 2: locally-connected stack ===========
                # ---- lconv3: 13x13 -> 9x9, 5x5 taps ----
                for p in range(81):
                    i, j = p // 9, p % 9
                    lwt = lwp.tile([128, 640], bf16, tag="lw3p")
                    nc.sync.dma_start(lwt[:], lw3p_d[p])
                    lws = lwp.tile([64, 320], bf16, tag="lw3s")
                    nc.sync.dma_start(lws[:], lw3s_d[p])
                    ps = lps.tile([64, 256], f32, tag="lps")
                    for q in range(10):        # (u, v0/v0+1) pairs, K=128
                        u, v0 = q // 2, 2 * (q % 2)
                        col = ((i + u) * 13 + (j + v0)) * 256
                        nc.tensor.matmul(ps[:], lwt[:, 64 * q:64 * (q + 1)],
                                         h2brep[:, col:col + 256],
                                         start=(q == 0), stop=False)
                    for s in range(5):         # (u, v=4) singles, K=64
                        col = ((i + s) * 13 + (j + 4)) * 256
                        nc.tensor.matmul(ps[:], lws[:, 64 * s:64 * (s + 1)],
                                         h2brep[0:64, col:col + 256],
                                         start=False, stop=(s == 4))
                    nc.scalar.activation(h3rep[0:64, 256 * p:256 * (p + 1)], ps[:],
                                         Tanh, bias=lb3t[:, p:p + 1])
                    if p >= 1:
                        nc.scalar.activation(
                            h3rep[64:128, 256 * (p - 1):256 * p], ps[:],
                            Tanh, bias=lb3t[:, p:p + 1])

                # ---- lconv4: 9x9 -> 5x5, 5x5 taps ----
                for p in range(25):
                    i, j = p // 5, p % 5
                    lwt = lwp.tile([128, 640], bf16, tag="lw4p")
                    nc.sync.dma_start(lwt[:], lw4p_d[p])
                    lws = lwp.tile([64, 320], bf16, tag="lw4s")
                    nc.sync.dma_start(lws[:], lw4s_d[p])
                    ps = lps.tile([64, 256], f32, tag="lps")
                    for q in range(10):
                        u, v0 = q // 2, 2 * (q % 2)
                        col = ((i + u) * 9 + (j + v0)) * 256
                        nc.tensor.matmul(ps[:], lwt[:, 64 * q:64 * (q + 1)],
                                         h3rep[:, col:col + 256],
                                         start=(q == 0), stop=False)
                    for s in range(5):
                        col = ((i + s) * 9 + (j + 4)) * 256
                        nc.tensor.matmul(ps[:], lws[:, 64 * s:64 * (s + 1)],
                                         h3rep[0:64, col:col + 256],
                                         start=False, stop=(s == 4))
                    nc.scalar.activation(h4rep[0:64, 256 * p:256 * (p + 1)], ps[:],
                                         Tanh, bias=lb4t[:, p:p + 1])
                    if p >= 1:
                        nc.scalar.activation(
                            h4rep[64:128, 256 * (p - 1):256 * p], ps[:],
                            Tanh, bias=lb4t[:, p:p + 1])

                # ---- lconv5: 5x5 -> 3x3, 3x3 taps ----
                for p in range(9):
                    i, j = p // 3, p % 3
                    lwt = lwp.tile([128, 192], bf16, tag="lw5p")
                    nc.sync.dma_start(lwt[:], lw5p_d[p])
                    lws = lwp.tile([64, 192], bf16, tag="lw5s")
                    nc.sync.dma_start(lws[:], lw5s_d[p])
                    ps = lps.tile([64, 256], f32, tag="lps")
                    for q in range(3):         # (u, v=0/1) pairs
                        col = ((i + q) * 5 + (j + 0)) * 256
                        nc.tensor.matmul(ps[:], lwt[:, 64 * q:64 * (q + 1)],
                                         h4rep[:, col:col + 256],
                                         start=(q == 0), stop=False)
                    for s in range(3):         # (u, v=2) singles
                        col = ((i + s) * 5 + (j + 2)) * 256
                        nc.tensor.matmul(ps[:], lws[:, 64 * s:64 * (s + 1)],
                                         h4rep[0:64, col:col + 256],
                                         start=False, stop=(s == 2))
                    nc.scalar.activation(h5t[:, 256 * p:256 * (p + 1)], ps[:],
                                         Tanh, bias=lb5t[:, p:p + 1])

                # ---- head: logits[o, b] = sum_f hw[o, f] feat[f, b] (h5 part) ----
                psh = hps.tile([2, 256], f32)
                for yx in range(9):
                    nc.tensor.matmul(psh[:], hwch[:, 2 * yx:2 * yx + 2],
                                     h5t[:, 256 * yx:256 * (yx + 1)],
                                     start=(yx == 0), stop=(yx == 8))
                lg = wp.tile([2, 256], f32)
                nc.vector.tensor_copy(lg[:], psh[:])
                nc.sync.dma_start(logits_d[:], lg[:])

    nc.compile()
    return nc


def _prep_weights(w1, b1, w2a, b2a, w2b, b2b, lw3, lb3, lw4, lb4, lw5, lb5, hw):
    """Host-side reshape of weights into the on-chip matmul layouts."""
    out = {}
    w1di = np.zeros((60, 3, 128), np.float32)
    for g in range(4):
        for di in range(3):
            for dj in range(3):
                w1di[15 * g + 5 * di:15 * g + 5 * di + 5, dj,
                     32 * g:32 * g + 32] = w1[:, :, di, dj].T   # [5ci, 32co]
    out["w1di"] = w1di.reshape(60, 3 * 128).astype(BF)
    out["b1t"] = np.tile(b1, 4)[:, None].astype(np.float32)

    w2adi = np.zeros((96, 3, 64), np.float32)
    for di in range(3):
        for dj in range(3):
            w2adi[32 * di:32 * di + 32, dj, :] = w2a[:, :, di, dj].T  # [32ci, 64co]
    out["w2adi"] = w2adi.reshape(96, 3 * 64).astype(BF)
    out["b2at"] = b2a[:, None].astype(np.float32)

    w2bbd = np.zeros((128, 9, 128), np.float32)
    for t, (di, dj) in enumerate(TAPS3):
        blk = w2b[:, :, di, dj].T         # [64ci, 64co]
        for g2 in range(2):
            w2bbd[64 * g2:64 * (g2 + 1), t, 64 * g2:64 * (g2 + 1)] = blk
    out["w2bbd"] = w2bbd.reshape(128, 9 * 128).astype(BF)
    out["b2bt"] = np.tile(b2b, 2)[:, None].astype(np.float32)

    def lc_pack(lw, Ho, Wo, kh, kw):
        npos = Ho * Wo
        npair = kh * (kw // 2)
        nsing = kh
        lp = np.zeros((npos, 128, npair * 64), np.float32)
        ls = np.zeros((npos, 64, nsing * 64), np.float32)
        for p in range(npos):
            i, j = p // Wo, p % Wo
            for q in range(npair):
                u, v0 = q // (kw // 2), 2 * (q % (kw // 2))
                lp[p, 0:64, 64 * q:64 * (q + 1)] = lw[i, j, :, :, u, v0].T
                lp[p, 64:128, 64 * q:64 * (q + 1)] = lw[i, j, :, :, u, v0 + 1].T
            for s in range(nsing):
                ls[p, 0:64, 64 * s:64 * (s + 1)] = lw[i, j, :, :, s, kw - 1].T
        return lp.astype(BF), ls.astype(BF)

    out["lw3p"], out["lw3s"] = lc_pack(lw3, 9, 9, 5, 5)
    out["lb3t"] = np.ascontiguousarray(
        lb3.transpose(2, 0, 1).reshape(64, 81)).astype(np.float32)
    out["lw4p"], out["lw4s"] = lc_pack(lw4, 5, 5, 5, 5)
    out["lb4t"] = np.ascontiguousarray(
        lb4.transpose(2, 0, 1).reshape(64, 25)).astype(np.float32)
    out["lw5p"], out["lw5s"] = lc_pack(lw5, 3, 3, 3, 3)
    out["lb5t"] = np.ascontiguousarray(
        lb5.transpose(2, 0, 1).reshape(64, 9)).astype(np.float32)

    # head: feature f = co*9 + yx; chunk yx -> [64co, 2]
    out["hwch"] = np.ascontiguousarray(
        hw[:, :576].reshape(2, 64, 9).transpose(1, 2, 0).reshape(64, 18)
    ).astype(BF)
    return out


def kernel(x, info, w1, b1, w2a, b2a, w2b, b2b, lw3, lb3, lw4, lb4, lw5, lb5,
           hw, hb, _trace=False):
    x = np.asarray(x, np.float32)
    if "nc" not in _CACHE:
        _CACHE["nc"] = _build_module()
    nc = _CACHE["nc"]

    wts = _prep_weights(np.asarray(w1, np.float32), np.asarray(b1, np.float32),
                        np.asarray(w2a, np.float32), np.asarray(b2a, np.float32),
                        np.asarray(w2b, np.float32), np.asarray(b2b, np.float32),
                        np.asarray(lw3, np.float32), np.asarray(lb3, np.float32),
                        np.asarray(lw4, np.float32), np.asarray(lb4, np.float32),
                        np.asarray(lw5, np.float32), np.asarray(lb5, np.float32),
                        np.asarray(hw, np.float32))

    # split rows by parity: plane 0 = even rows (30), plane 1 = odd rows (30)
    xr = x.reshape(B_FULL, 5, 30, 2, 60)
    xb = np.ascontiguousarray(
        np.stack([xr[:, :, :, 0, :], xr[:, :, :, 1, :]], axis=1)
        .reshape(B_FULL, 2, 5, 1800)).astype(BF)
    in_maps = []
    for c in range(N_CORES):
        m = dict(wts)
        m["x"] = xb[c * B_CORE:(c + 1) * B_CORE]
        in_maps.append(m)

    res = bass_utils.run_bass_kernel_spmd(
        nc, in_maps, core_ids=list(range(N_CORES)), trace=_trace)
    if _trace:
        _CACHE["last_results"] = res

    logits = np.concatenate(
        [res.results[c]["logits"].T for c in range(N_CORES)], axis=0)  # [2048, 2]

    # host-side tail: info contribution + bias + softmax (fp32)
    info = np.asarray(info, np.float32)
    hw = np.asarray(hw, np.float32)
    hb = np.asarray(hb, np.float32)
    logits = logits + info @ hw[:, 576:].T + hb[None, :]
    m = logits.max(axis=1, keepdims=True)
    e = np.exp(logits - m)
    return (e / e.sum(axis=1, keepdims=True)).astype(np.float32)


# revision 9
# speedup vs baseline: 1.2588x; 1.0955x over previous
"""Trainium2 Bass kernel for the DeepFace-style CNN (nn_DeepFace_10574209482846).

Sharding: pure data parallel — batch 2048 split as 256 images per core
across 8 cores; all weights replicated (host-preprocessed into matmul-
friendly layouts, cast to bf16).

Per-core layout: 256 images = 4 groups of 64; sub-batches of 8 images
(2 per group).  Dense convs contract the 3 row-taps (di) inside K via
physically row-shifted input stacks (built by DMA for conv1, by cheap
contiguous DVE copies for conv2a); the 3 column-taps (dj) are free-dim
offsets.  conv2b keeps the 2-group block-diagonal K=128 form (already
optimal).  Locally-connected layers use a batch-contiguous (y, x, b)
layout with a +1-column-shifted replica on partitions 64..127 (written
directly by dual scalar activations) so two kernel taps contract per
matmul (K=128).
"""

import numpy as np
import concourse.bass as bass
import concourse.bacc as bacc
import concourse.tile as tile
import concourse.mybir as mybir
from concourse import bass_utils

bf16 = mybir.dt.bfloat16
f32 = mybir.dt.float32
BF = mybir.dt.np(bf16)  # ml_dtypes.bfloat16

N_CORES = 8
B_FULL = 2048
B_CORE = 256          # images per core
SB = 8                # images per sub-batch (2 per group)
NSB = B_CORE // SB    # 32
BSB = SB // 4         # 2 images per group per sub-batch

TAPS3 = [(di, dj) for di in range(3) for dj in range(3)]

_CACHE = {}


def _build_module(nsb=NSB, phase2=True):
    nc = bacc.Bacc("TRN2", target_bir_lowering=False, debug=False,
                   enable_asserts=True, num_devices=N_CORES)

    # ---- DRAM I/O ----
    # x pre-split on host into row-parity planes: plane 0 = even rows
    # 0,2,..,58 (30), plane 1 = odd rows 1,3,..,59 (30).
    x_d = nc.dram_tensor("x", [B_CORE, 2, 5, 1800], bf16,
                         kind="ExternalInput").ap()
    w1di_d = nc.dram_tensor("w1di", [60, 3 * 128], bf16, kind="ExternalInput").ap()
    b1t_d = nc.dram_tensor("b1t", [128, 1], f32, kind="ExternalInput").ap()
    w2adi_d = nc.dram_tensor("w2adi", [96, 3 * 64], bf16, kind="ExternalInput").ap()
    b2at_d = nc.dram_tensor("b2at", [128, 1], f32, kind="ExternalInput").ap()
    w2bbd_d = nc.dram_tensor("w2bbd", [128, 9 * 128], bf16, kind="ExternalInput").ap()
    b2bt_d = nc.dram_tensor("b2bt", [128, 1], f32, kind="ExternalInput").ap()
    lw3p_d = nc.dram_tensor("lw3p", [81, 128, 640], bf16, kind="ExternalInput").ap()
    lw3s_d = nc.dram_tensor("lw3s", [81, 64, 320], bf16, kind="ExternalInput").ap()
    lb3_d = nc.dram_tensor("lb3t", [64, 81], f32, kind="ExternalInput").ap()
    lw4p_d = nc.dram_tensor("lw4p", [25, 128, 640], bf16, kind="ExternalInput").ap()
    lw4s_d = nc.dram_tensor("lw4s", [25, 64, 320], bf16, kind="ExternalInput").ap()
    lb4_d = nc.dram_tensor("lb4t", [64, 25], f32, kind="ExternalInput").ap()
    lw5p_d = nc.dram_tensor("lw5p", [9, 128, 192], bf16, kind="ExternalInput").ap()
    lw5s_d = nc.dram_tensor("lw5s", [9, 64, 192], bf16, kind="ExternalInput").ap()
    lb5_d = nc.dram_tensor("lb5t", [64, 9], f32, kind="ExternalInput").ap()
    hwch_d = nc.dram_tensor("hwch", [64, 18], bf16, kind="ExternalInput").ap()
    logits_d = nc.dram_tensor("logits", [2, B_CORE], f32, kind="ExternalOutput").ap()

    Tanh = mybir.ActivationFunctionType.Tanh

    with tile.TileContext(nc) as tc:
        with (
            tc.tile_pool(name="wp", bufs=1) as wp,
            tc.tile_pool(name="lwp", bufs=2) as lwp,
            tc.tile_pool(name="xp", bufs=2) as xp,
            tc.tile_pool(name="h1p", bufs=1) as h1p,
            tc.tile_pool(name="stkp", bufs=2) as stkp,
            tc.tile_pool(name="h2ap", bufs=1) as h2ap,
            tc.tile_pool(name="bigp", bufs=1) as bigp,
            tc.tile_pool(name="cps", bufs=4, space="PSUM") as cps,
            tc.tile_pool(name="lps", bufs=3, space="PSUM") as lps,
            tc.tile_pool(name="hps", bufs=1, space="PSUM") as hps,
        ):
            # ---- persistent weights ----
            w1di = wp.tile([60, 3 * 128], bf16)
            nc.sync.dma_start(w1di[:], w1di_d[:])
            b1t = wp.tile([128, 1], f32)
            nc.sync.dma_start(b1t[:], b1t_d[:])
            w2adi = wp.tile([96, 3 * 64], bf16)
            nc.sync.dma_start(w2adi[:], w2adi_d[:])
            b2at = wp.tile([128, 1], f32)
            nc.sync.dma_start(b2at[:], b2at_d[:])
            w2bbd = wp.tile([128, 9 * 128], bf16)
            nc.sync.dma_start(w2bbd[:], w2bbd_d[:])
            b2bt = wp.tile([128, 1], f32)
            nc.sync.dma_start(b2bt[:], b2bt_d[:])
            lb3t = wp.tile([64, 81], f32)
            nc.sync.dma_start(lb3t[:], lb3_d[:])
            lb4t = wp.tile([64, 25], f32)
            nc.sync.dma_start(lb4t[:], lb4_d[:])
            lb5t = wp.tile([64, 9], f32)
            nc.sync.dma_start(lb5t[:], lb5_d[:])
            hwch = wp.tile([64, 18], bf16)
            nc.sync.dma_start(hwch[:], hwch_d[:])

            # ---- persistent activations (batch-contiguous, (y, x, b)) ----
            h2brep = bigp.tile([128, 169 * 256], bf16)   # rows 0-63 h2b, 64-127 +1col
            h3rep = bigp.tile([128, 81 * 256], bf16)
            h4rep = bigp.tile([128, 25 * 256], bf16)
            h5t = bigp.tile([64, 9 * 256], bf16)

            h2bv = h2brep[:].rearrange("c (y x b) -> c b y x", y=13, x=13, b=256)

            # ================= phase 1: conv1 -> conv2a -> conv2b =============
            # software-pipelined: iter sb runs conv1(sb)+copies(sb), then
            # conv2a(sb-1)+conv2b(sb-1), so DVE copies hide under PE work.
            C1STRIPS = [(0, 8), (8, 8), (16, 8), (24, 5)]
            C2STRIPS = [(0, 9), (9, 9), (18, 9)]

            def load_x(sb):
                # row-shifted di-stack: partition (g, di, c) = 15g+5di+c,
                # free (b, y, x) with tile row y = input row 2y+di.
                xs = xp.tile([60, BSB * 29 * 60], bf16, tag="xs", name="xs")
                xsb = xs[:].rearrange("p (b m) -> p b m", b=BSB)
                for g in range(4):
                    b0 = 64 * g + BSB * sb
                    base = x_d[b0:b0 + BSB]          # [BSB, 2, 5, 1800]
                    for di, (pl, r0) in enumerate([(0, 0), (1, 0), (0, 1)]):
                        src = base[:, pl, :, 60 * r0:60 * r0 + 1740].rearrange(
                            "b c m -> c b m")
                        nc.sync.dma_start(
                            xsb[15 * g + 5 * di:15 * g + 5 * di + 5, :, :], src)
                return xs

            def conv1(xs):
                xsv = xs[:].rearrange("p (b y x) -> p b y x", b=BSB, y=29, x=60)
                h1 = h1p.tile([128, BSB * 841], bf16, tag="h1", name="h1")
                h1v = h1[:].rearrange("c (b h w) -> c b h w", b=BSB, h=29, w=29)
                for (y0, ny) in C1STRIPS:
                    ps = cps.tile([128, 486], f32, tag="cps", name="c1ps")
                    psw = ps[:, :BSB * ny * 29]
                    for dj in range(3):
                        rhs = xsv[:, :, y0:y0 + ny, dj:dj + 57:2]
                        nc.tensor.matmul(psw, w1di[:, 128 * dj:128 * (dj + 1)],
                                         rhs, start=(dj == 0), stop=(dj == 2))
                    nc.scalar.activation(h1v[:, :, y0:y0 + ny, :], psw, Tanh,
                                         bias=b1t[:])
                # di-stack for conv2a: per group [96 = 3di x 32c, (b y x)],
                # di shift = +29 elements (one y row), flat contiguous copy.
                stk = stkp.tile([96, 4 * 1682], bf16, tag="stk", name="stk")
                for g in range(4):
                    for di in range(3):
                        L = 1682 - 29 * di
                        nc.vector.tensor_copy(
                            stk[32 * di:32 * di + 32, 1682 * g:1682 * g + L],
                            h1[32 * g:32 * g + 32, 29 * di:29 * di + L])
                return stk

            def conv23(stk, sb):
                # ---- conv2a: per group, K=96 (3 di x 32 ci), dj by offset ----
                h2a_t = {}
                for r in range(2):
                    h2a_t[r] = h2ap.tile([128, BSB * 729], bf16, tag=f"h2a{r}",
                                         name=f"h2a{r}")
                for r in range(2):
                    h2av = h2a_t[r][:].rearrange("c (b h w) -> c b h w",
                                                 b=BSB, h=27, w=27)
                    for (y0, ny) in C2STRIPS:
                        ps = cps.tile([128, 486], f32, tag="cps", name="c2aps")
                        for half in range(2):
                            g = 2 * r + half
                            sv = stk[:, 1682 * g:1682 * (g + 1)].rearrange(
                                "p (b h w) -> p b h w", b=BSB, h=29, w=29)
                            psw = ps[64 * half:64 * half + 64, :BSB * ny * 27]
                            for dj in range(3):
                                rhs = sv[:, :, y0:y0 + ny, dj:dj + 27]
                                nc.tensor.matmul(
                                    psw, w2adi[:, 64 * dj:64 * (dj + 1)], rhs,
                                    start=(dj == 0), stop=(dj == 2))
                        nc.scalar.activation(
                            h2av[:, :, y0:y0 + ny, :],
                            ps[:, :BSB * ny * 27], Tanh, bias=b2at[:])

                # ---- conv2b: per pair, K=128 block-diag, stride 2 ----
                for r in range(2):
                    h2av = h2a_t[r][:].rearrange("c (b h w) -> c b h w",
                                                 b=BSB, h=27, w=27)
                    ps = cps.tile([128, 486], f32, tag="cps", name="c2bps")
                    psw = ps[:, :BSB * 169]
                    for t, (di, dj) in enumerate(TAPS3):
                        rhs = h2av[:, :, di: di + 25: 2, dj: dj + 25: 2]
                        nc.tensor.matmul(psw, w2bbd[:, 128 * t:128 * (t + 1)],
                                         rhs, start=(t == 0), stop=(t == 8))
                    # evacuate: psum rows (g-even 0:64 / g-odd 64:128) into
                    # h2brep[0:64, (y, x, b)] plus the +1-col replica rows.
                    psv = psw.rearrange("c (b y x) -> c b y x", b=BSB, y=13, x=13)
                    psf = psw.rearrange("c (b yx) -> c b yx", b=BSB, yx=169)
                    for g2 in range(2):
                        gb = 64 * (2 * r + g2) + BSB * sb
                        nc.scalar.activation(
                            h2bv[0:64, gb:gb + BSB, :, :],
                            psv[64 * g2:64 * (g2 + 1), :, :, :], Tanh,
                            bias=b2bt[64 * g2:64 * (g2 + 1)])
                        rep = h2brep[64:128, gb:gb + 168 * 256].rearrange(
                            "c (yx b) -> c b yx", b=256)[:, 0:BSB, :]
                        nc.scalar.activation(
                            rep, psf[64 * g2:64 * (g2 + 1), :, 1:169], Tanh,
                            bias=b2bt[64 * g2:64 * (g2 + 1)])

            xs_cur = load_x(0)
            stk_prev = None
            for sb in range(nsb):
                stk_cur = conv1(xs_cur)
                if sb + 1 < nsb:
                    xs_cur = load_x(sb + 1)
                if stk_prev is not None:
                    conv23(stk_prev, sb - 1)
                stk_prev = stk_cur
            conv23(stk_prev, nsb - 1)

            if not phase2:
                lg = wp.tile([2, 256], f32, name="lg")
                nc.vector.tensor_copy(lg[:], h2brep[0:2, 0:512].bitcast(f32))
                nc.sync.dma_start(logits_d[:], lg[:])
            if phase2:
                # ================= phase 2: locally-connected stack ===========
                # ---- lconv3: 13x13 -> 9x9, 5x5 taps ----
                for p in range(81):
                    i, j = p // 9, p % 9
                    lwt = lwp.tile([128, 640], bf16, tag="lw3p")
                    nc.sync.dma_start(lwt[:], lw3p_d[p])
                    lws = lwp.tile([64, 320], bf16, tag="lw3s")
                    nc.sync.dma_start(lws[:], lw3s_d[p])
                    ps = lps.tile([64, 256], f32, tag="lps")
                    for q in range(10):        # (u, v0/v0+1) pairs, K=128
                        u, v0 = q // 2, 2 * (q % 2)
                        col = ((i + u) * 13 + (j + v0)) * 256
                        nc.tensor.matmul(ps[:], lwt[:, 64 * q:64 * (q + 1)],
                                         h2brep[:, col:col + 256],
                                         start=(q == 0), stop=False)
                    for s in range(5):         # (u, v=4) singles, K=64
                        col = ((i + s) * 13 + (j + 4)) * 256
                        nc.tensor.matmul(ps[:], lws[:, 64 * s:64 * (s + 1)],
                                         h2brep[0:64, col:col + 256],
                                         start=False, stop=(s == 4))
                    nc.scalar.activation(h3rep[0:64, 256 * p:256 * (p + 1)], ps[:],
                                         Tanh, bias=lb3t[:, p:p + 1])
                    if p >= 1:
                        nc.scalar.activation(
                            h3rep[64:128, 256 * (p - 1):256 * p], ps[:],
                            Tanh, bias=lb3t[:, p:p + 1])

                # ---- lconv4: 9x9 -> 5x5, 5x5 taps ----
                for p in range(25):
                    i, j = p // 5, p % 5
                    lwt = lwp.tile([128, 640], bf16, tag="lw4p")
                    nc.sync.dma_start(lwt[:], lw4p_d[p])
                    lws = lwp.tile([64, 320], bf16, tag="lw4s")
                    nc.sync.dma_start(lws[:], lw4s_d[p])
                    ps = lps.tile([64, 256], f32, tag="lps")
                    for q in range(10):
                        u, v0 = q // 2, 2 * (q % 2)
                        col = ((i + u) * 9 + (j + v0)) * 256
                        nc.tensor.matmul(ps[:], lwt[:, 64 * q:64 * (q + 1)],
                                         h3rep[:, col:col + 256],
                                         start=(q == 0), stop=False)
                    for s in range(5):
                        col = ((i + s) * 9 + (j + 4)) * 256
                        nc.tensor.matmul(ps[:], lws[:, 64 * s:64 * (s + 1)],
                                         h3rep[0:64, col:col + 256],
                                         start=False, stop=(s == 4))
                    nc.scalar.activation(h4rep[0:64, 256 * p:256 * (p + 1)], ps[:],
                                         Tanh, bias=lb4t[:, p:p + 1])
                    if p >= 1:
                        nc.scalar.activation(
                            h4rep[64:128, 256 * (p - 1):256 * p], ps[:],
                            Tanh, bias=lb4t[:, p:p + 1])

                # ---- lconv5: 5x5 -> 3x3, 3x3 taps ----
                for p in range(9):
                    i, j = p // 3, p % 3
                    lwt = lwp.tile([128, 192], bf16, tag="lw5p")
                    nc.sync.dma_start(lwt[:], lw5p_d[p])
                    lws = lwp.tile([64, 192], bf16, tag="lw5s")
                    nc.sync.dma_start(lws[:], lw5s_d[p])
                    ps = lps.tile([64, 256], f32, tag="lps")
                    for q in range(3):         # (u, v=0/1) pairs
                        col = ((i + q) * 5 + (j + 0)) * 256
                        nc.tensor.matmul(ps[:], lwt[:, 64 * q:64 * (q + 1)],
                                         h4rep[:, col:col + 256],
                                         start=(q == 0), stop=False)
                    for s in range(3):         # (u, v=2) singles
                        col = ((i + s) * 5 + (j + 2)) * 256
                        nc.tensor.matmul(ps[:], lws[:, 64 * s:64 * (s + 1)],
                                         h4rep[0:64, col:col + 256],
                                         start=False, stop=(s == 2))
                    nc.scalar.activation(h5t[:, 256 * p:256 * (p + 1)], ps[:],
                                         Tanh, bias=lb5t[:, p:p + 1])

                # ---- head: logits[o, b] = sum_f hw[o, f] feat[f, b] (h5 part) ----
                psh = hps.tile([2, 256], f32)
                for yx in range(9):
                    nc.tensor.matmul(psh[:], hwch[:, 2 * yx:2 * yx + 2],
                                     h5t[:, 256 * yx:256 * (yx + 1)],
                                     start=(yx == 0), stop=(yx == 8))
                lg = wp.tile([2, 256], f32)
                nc.vector.tensor_copy(lg[:], psh[:])
                nc.sync.dma_start(logits_d[:], lg[:])

    nc.compile()
    return nc


def _prep_weights(w1, b1, w2a, b2a, w2b, b2b, lw3, lb3, lw4, lb4, lw5, lb5, hw):
    """Host-side reshape of weights into the on-chip matmul layouts."""
    out = {}
    w1di = np.zeros((60, 3, 128), np.float32)
    for g in range(4):
        for di in range(3):
            for dj in range(3):
                w1di[15 * g + 5 * di:15 * g + 5 * di + 5, dj,
                     32 * g:32 * g + 32] = w1[:, :, di, dj].T   # [5ci, 32co]
    out["w1di"] = w1di.reshape(60, 3 * 128).astype(BF)
    out["b1t"] = np.tile(b1, 4)[:, None].astype(np.float32)

    w2adi = np.zeros((96, 3, 64), np.float32)
    for di in range(3):
        for dj in range(3):
            w2adi[32 * di:32 * di + 32, dj, :] = w2a[:, :, di, dj].T  # [32ci, 64co]
    out["w2adi"] = w2adi.reshape(96, 3 * 64).astype(BF)
    out["b2at"] = np.tile(b2a, 2)[:, None].astype(np.float32)

    w2bbd = np.zeros((128, 9, 128), np.float32)
    for t, (di, dj) in enumerate(TAPS3):
        blk = w2b[:, :, di, dj].T         # [64ci, 64co]
        for g2 in range(2):
            w2bbd[64 * g2:64 * (g2 + 1), t, 64 * g2:64 * (g2 + 1)] = blk
    out["w2bbd"] = w2bbd.reshape(128, 9 * 128).astype(BF)
    out["b2bt"] = np.tile(b2b, 2)[:, None].astype(np.float32)

    def lc_pack(lw, Ho, Wo, kh, kw):
        npos = Ho * Wo
        npair = kh * (kw // 2)
        nsing = kh
        lp = np.zeros((npos, 128, npair * 64), np.float32)
        ls = np.zeros((npos, 64, nsing * 64), np.float32)
        for p in range(npos):
            i, j = p // Wo, p % Wo
            for q in range(npair):
                u, v0 = q // (kw // 2), 2 * (q % (kw // 2))
                lp[p, 0:64, 64 * q:64 * (q + 1)] = lw[i, j, :, :, u, v0].T
                lp[p, 64:128, 64 * q:64 * (q + 1)] = lw[i, j, :, :, u, v0 + 1].T
            for s in range(nsing):
                ls[p, 0:64, 64 * s:64 * (s + 1)] = lw[i, j, :, :, s, kw - 1].T
        return lp.astype(BF), ls.astype(BF)

    out["lw3p"], out["lw3s"] = lc_pack(lw3, 9, 9, 5, 5)
    out["lb3t"] = np.ascontiguousarray(
        lb3.transpose(2, 0, 1).reshape(64, 81)).astype(np.float32)
    out["lw4p"], out["lw4s"] = lc_pack(lw4, 5, 5, 5, 5)
    out["lb4t"] = np.ascontiguousarray(
        lb4.transpose(2, 0, 1).reshape(64, 25)).astype(np.float32)
    out["lw5p"], out["lw5s"] = lc_pack(lw5, 3, 3, 3, 3)
    out["lb5t"] = np.ascontiguousarray(
        lb5.transpose(2, 0, 1).reshape(64, 9)).astype(np.float32)

    # head: feature f = co*9 + yx; chunk yx -> [64co, 2]
    out["hwch"] = np.ascontiguousarray(
        hw[:, :576].reshape(2, 64, 9).transpose(1, 2, 0).reshape(64, 18)
    ).astype(BF)
    return out


def kernel(x, info, w1, b1, w2a, b2a, w2b, b2b, lw3, lb3, lw4, lb4, lw5, lb5,
           hw, hb, _trace=False):
    x = np.asarray(x, np.float32)
    if "nc" not in _CACHE:
        _CACHE["nc"] = _build_module()
    nc = _CACHE["nc"]

    wts = _prep_weights(np.asarray(w1, np.float32), np.asarray(b1, np.float32),
                        np.asarray(w2a, np.float32), np.asarray(b2a, np.float32),
                        np.asarray(w2b, np.float32), np.asarray(b2b, np.float32),
                        np.asarray(lw3, np.float32), np.asarray(lb3, np.float32),
                        np.asarray(lw4, np.float32), np.asarray(lb4, np.float32),
                        np.asarray(lw5, np.float32), np.asarray(lb5, np.float32),
                        np.asarray(hw, np.float32))

    # split rows by parity: plane 0 = even rows (30), plane 1 = odd rows (30)
    xr = x.reshape(B_FULL, 5, 30, 2, 60)
    xb = np.ascontiguousarray(
        np.stack([xr[:, :, :, 0, :], xr[:, :, :, 1, :]], axis=1)
        .reshape(B_FULL, 2, 5, 1800)).astype(BF)
    in_maps = []
    for c in range(N_CORES):
        m = dict(wts)
        m["x"] = xb[c * B_CORE:(c + 1) * B_CORE]
        in_maps.append(m)

    res = bass_utils.run_bass_kernel_spmd(
        nc, in_maps, core_ids=list(range(N_CORES)), trace=_trace)
    if _trace:
        _CACHE["last_results"] = res

    logits = np.concatenate(
        [res.results[c]["logits"].T for c in range(N_CORES)], axis=0)  # [2048, 2]

    # host-side tail: info contribution + bias + softmax (fp32)
    info = np.asarray(info, np.float32)
    hw = np.asarray(hw, np.float32)
    hb = np.asarray(hb, np.float32)
    logits = logits + info @ hw[:, 576:].T + hb[None, :]
    m = logits.max(axis=1, keepdims=True)
    e = np.exp(logits - m)
    return (e / e.sum(axis=1, keepdims=True)).astype(np.float32)


# revision 11
# speedup vs baseline: 1.6173x; 1.2848x over previous
"""Trainium2 Bass kernel for the DeepFace-style CNN (nn_DeepFace_10574209482846).

Sharding: pure data parallel — batch 2048 split as 256 images per core
across 8 cores; all weights replicated (host-preprocessed into matmul-
friendly layouts, cast to bf16).

Per-core layout: 256 images = 4 groups of 64; sub-batches of 8 images
(2 per group).  Dense convs contract the 3 row-taps (di) inside K via
physically row-shifted input stacks (built by DMA for conv1, by cheap
contiguous DVE copies for conv2a); the 3 column-taps (dj) are free-dim
offsets.  conv2b keeps the 2-group block-diagonal K=128 form (already
optimal).  Locally-connected layers use a batch-contiguous (y, x, b)
layout with a +1-column-shifted replica on partitions 64..127 (written
directly by dual scalar activations) so two kernel taps contract per
matmul (K=128).
"""

import numpy as np
import concourse.bass as bass
import concourse.bacc as bacc
import concourse.tile as tile
import concourse.mybir as mybir
from concourse import bass_utils

bf16 = mybir.dt.bfloat16
f32 = mybir.dt.float32
BF = mybir.dt.np(bf16)  # ml_dtypes.bfloat16

N_CORES = 8
B_FULL = 2048
B_CORE = 256          # images per core
SB = 8                # images per sub-batch (2 per group)
NSB = B_CORE // SB    # 32
BSB = SB // 4         # 2 images per group per sub-batch

TAPS3 = [(di, dj) for di in range(3) for dj in range(3)]

_CACHE = {}


def _build_module(nsb=NSB, phase2=True):
    nc = bacc.Bacc("TRN2", target_bir_lowering=False, debug=False,
                   enable_asserts=True, num_devices=N_CORES)

    # ---- DRAM I/O ----
    # x pre-split on host into row-parity planes: plane 0 = even rows
    # 0,2,..,58 (30), plane 1 = odd rows 1,3,..,59 (30).
    x_d = nc.dram_tensor("x", [B_CORE, 2, 5, 1800], bf16,
                         kind="ExternalInput").ap()
    w1di_d = nc.dram_tensor("w1di", [60, 3 * 128], bf16, kind="ExternalInput").ap()
    b1t_d = nc.dram_tensor("b1t", [128, 1], f32, kind="ExternalInput").ap()
    w2adi_d = nc.dram_tensor("w2adi", [96, 3 * 64], bf16, kind="ExternalInput").ap()
    b2at_d = nc.dram_tensor("b2at", [128, 1], f32, kind="ExternalInput").ap()
    w2bbd_d = nc.dram_tensor("w2bbd", [128, 9 * 128], bf16, kind="ExternalInput").ap()
    b2bt_d = nc.dram_tensor("b2bt", [128, 1], f32, kind="ExternalInput").ap()
    lw3p_d = nc.dram_tensor("lw3p", [81, 128, 640], bf16, kind="ExternalInput").ap()
    lw3s_d = nc.dram_tensor("lw3s", [81, 64, 320], bf16, kind="ExternalInput").ap()
    lb3_d = nc.dram_tensor("lb3t", [64, 81], f32, kind="ExternalInput").ap()
    lw4p_d = nc.dram_tensor("lw4p", [25, 128, 640], bf16, kind="ExternalInput").ap()
    lw4s_d = nc.dram_tensor("lw4s", [25, 64, 320], bf16, kind="ExternalInput").ap()
    lb4_d = nc.dram_tensor("lb4t", [64, 25], f32, kind="ExternalInput").ap()
    lw5p_d = nc.dram_tensor("lw5p", [9, 128, 192], bf16, kind="ExternalInput").ap()
    lw5s_d = nc.dram_tensor("lw5s", [9, 64, 192], bf16, kind="ExternalInput").ap()
    lb5_d = nc.dram_tensor("lb5t", [64, 9], f32, kind="ExternalInput").ap()
    hwch_d = nc.dram_tensor("hwch", [64, 18], bf16, kind="ExternalInput").ap()
    logits_d = nc.dram_tensor("logits", [2, B_CORE], f32, kind="ExternalOutput").ap()

    Tanh = mybir.ActivationFunctionType.Tanh

    with tile.TileContext(nc) as tc:
        with (
            tc.tile_pool(name="wp", bufs=1) as wp,
            tc.tile_pool(name="lwp", bufs=2) as lwp,
            tc.tile_pool(name="xp", bufs=2) as xp,
            tc.tile_pool(name="h1p", bufs=1) as h1p,
            tc.tile_pool(name="stkp", bufs=2) as stkp,
            tc.tile_pool(name="h2ap", bufs=1) as h2ap,
            tc.tile_pool(name="bigp", bufs=1) as bigp,
            tc.tile_pool(name="cps", bufs=4, space="PSUM") as cps,
            tc.tile_pool(name="lps", bufs=3, space="PSUM") as lps,
            tc.tile_pool(name="hps", bufs=1, space="PSUM") as hps,
        ):
            # ---- persistent weights ----
            w1di = wp.tile([60, 3 * 128], bf16)
            nc.sync.dma_start(w1di[:], w1di_d[:])
            b1t = wp.tile([128, 1], f32)
            nc.sync.dma_start(b1t[:], b1t_d[:])
            w2adi = wp.tile([96, 3 * 64], bf16)
            nc.sync.dma_start(w2adi[:], w2adi_d[:])
            b2at = wp.tile([128, 1], f32)
            nc.sync.dma_start(b2at[:], b2at_d[:])
            w2bbd = wp.tile([128, 9 * 128], bf16)
            nc.sync.dma_start(w2bbd[:], w2bbd_d[:])
            b2bt = wp.tile([128, 1], f32)
            nc.sync.dma_start(b2bt[:], b2bt_d[:])
            lb3t = wp.tile([64, 81], f32)
            nc.sync.dma_start(lb3t[:], lb3_d[:])
            lb4t = wp.tile([64, 25], f32)
            nc.sync.dma_start(lb4t[:], lb4_d[:])
            lb5t = wp.tile([64, 9], f32)
            nc.sync.dma_start(lb5t[:], lb5_d[:])
            hwch = wp.tile([64, 18], bf16)
            nc.sync.dma_start(hwch[:], hwch_d[:])

            # ---- persistent activations (batch-contiguous, (y, x, b)) ----
            h2brep = bigp.tile([128, 169 * 256], bf16)   # rows 0-63 h2b, 64-127 +1col
            h3rep = bigp.tile([128, 81 * 256], bf16)
            h4rep = bigp.tile([128, 25 * 256], bf16)
            h5t = bigp.tile([64, 9 * 256], bf16)

            h2bv = h2brep[:].rearrange("c (y x b) -> c b y x", y=13, x=13, b=256)

            # ================= phase 1: conv1 -> conv2a -> conv2b =============
            # software-pipelined: iter sb runs conv1(sb)+copies(sb), then
            # conv2a(sb-1)+conv2b(sb-1), so DVE copies hide under PE work.
            C1STRIPS = [(0, 8), (8, 8), (16, 8), (24, 5)]
            C2STRIPS = [(0, 9), (9, 9), (18, 9)]

            def load_x(sb):
                # row-shifted di-stack: partition (g, di, c) = 15g+5di+c,
                # free (b, y, x) with tile row y = input row 2y+di.
                xs = xp.tile([60, BSB * 29 * 60], bf16, tag="xs", name="xs")
                xsb = xs[:].rearrange("p (b m) -> p b m", b=BSB)
                for g in range(4):
                    b0 = 64 * g + BSB * sb
                    base = x_d[b0:b0 + BSB]          # [BSB, 2, 5, 1800]
                    for di, (pl, r0) in enumerate([(0, 0), (1, 0), (0, 1)]):
                        src = base[:, pl, :, 60 * r0:60 * r0 + 1740].rearrange(
                            "b c m -> c b m")
                        nc.sync.dma_start(
                            xsb[15 * g + 5 * di:15 * g + 5 * di + 5, :, :], src)
                return xs

            def conv1(xs):
                xsv = xs[:].rearrange("p (b y x) -> p b y x", b=BSB, y=29, x=60)
                h1 = h1p.tile([128, BSB * 841], bf16, tag="h1", name="h1")
                h1v = h1[:].rearrange("c (b h w) -> c b h w", b=BSB, h=29, w=29)
                for (y0, ny) in C1STRIPS:
                    ps = cps.tile([128, 486], f32, tag="cps", name="c1ps")
                    psw = ps[:, :BSB * ny * 29]
                    for dj in range(3):
                        rhs = xsv[:, :, y0:y0 + ny, dj:dj + 57:2]
                        nc.tensor.matmul(psw, w1di[:, 128 * dj:128 * (dj + 1)],
                                         rhs, start=(dj == 0), stop=(dj == 2))
                    nc.scalar.activation(h1v[:, :, y0:y0 + ny, :], psw, Tanh,
                                         bias=b1t[:])
                # di-stack for conv2a: per group [96 = 3di x 32c, (b y x)],
                # di shift = +29 elements (one y row), flat contiguous copy.
                stk = stkp.tile([96, 4 * 1682], bf16, tag="stk", name="stk")
                for g in range(4):
                    for di in range(3):
                        L = 1682 - 29 * di
                        nc.vector.tensor_copy(
                            stk[32 * di:32 * di + 32, 1682 * g:1682 * g + L],
                            h1[32 * g:32 * g + 32, 29 * di:29 * di + L])
                return stk

            def conv23(stk, sb):
                # ---- conv2a: per group, K=96 (3 di x 32 ci), dj by offset ----
                h2a_t = {}
                for r in range(2):
                    h2a_t[r] = h2ap.tile([128, BSB * 729], bf16, tag=f"h2a{r}",
                                         name=f"h2a{r}")
                for r in range(2):
                    h2av = h2a_t[r][:].rearrange("c (b h w) -> c b h w",
                                                 b=BSB, h=27, w=27)
                    for (y0, ny) in C2STRIPS:
                        ps = cps.tile([128, 486], f32, tag="cps", name="c2aps")
                        for half in range(2):
                            g = 2 * r + half
                            sv = stk[:, 1682 * g:1682 * (g + 1)].rearrange(
                                "p (b h w) -> p b h w", b=BSB, h=29, w=29)
                            psw = ps[64 * half:64 * half + 64, :BSB * ny * 27]
                            for dj in range(3):
                                rhs = sv[:, :, y0:y0 + ny, dj:dj + 27]
                                nc.tensor.matmul(
                                    psw, w2adi[:, 64 * dj:64 * (dj + 1)], rhs,
                                    start=(dj == 0), stop=(dj == 2))
                        nc.scalar.activation(
                            h2av[:, :, y0:y0 + ny, :],
                            ps[:, :BSB * ny * 27], Tanh, bias=b2at[:])

                # ---- conv2b: per pair, K=128 block-diag, stride 2 ----
                for r in range(2):
                    h2av = h2a_t[r][:].rearrange("c (b h w) -> c b h w",
                                                 b=BSB, h=27, w=27)
                    ps = cps.tile([128, 486], f32, tag="cps", name="c2bps")
                    psw = ps[:, :BSB * 169]
                    for t, (di, dj) in enumerate(TAPS3):
                        rhs = h2av[:, :, di: di + 25: 2, dj: dj + 25: 2]
                        nc.tensor.matmul(psw, w2bbd[:, 128 * t:128 * (t + 1)],
                                         rhs, start=(t == 0), stop=(t == 8))
                    # evacuate: psum rows (g-even 0:64 / g-odd 64:128) into
                    # h2brep[0:64, (y, x, b)] plus the +1-col replica rows.
                    psv = psw.rearrange("c (b y x) -> c b y x", b=BSB, y=13, x=13)
                    for g2 in range(2):
                        gb = 64 * (2 * r + g2) + BSB * sb
                        nc.scalar.activation(
                            h2bv[0:64, gb:gb + BSB, :, :],
                            psv[64 * g2:64 * (g2 + 1), :, :, :], Tanh,
                            bias=b2bt[64 * g2:64 * (g2 + 1)])
                        # +1-col replica rows: pure copy on the (idle) DVE
                        rep = h2brep[64:128, 0:168 * 256].rearrange(
                            "c (yx b) -> c b yx", b=256)[:, gb:gb + BSB, :]
                        srcr = h2brep[0:64, 256:256 + 168 * 256].rearrange(
                            "c (yx b) -> c b yx", b=256)[:, gb:gb + BSB, :]
                        nc.vector.tensor_copy(rep, srcr)

            xs_cur = load_x(0)
            stk_prev = None
            for sb in range(nsb):
                stk_cur = conv1(xs_cur)
                if sb + 1 < nsb:
                    xs_cur = load_x(sb + 1)
                if stk_prev is not None:
                    conv23(stk_prev, sb - 1)
                stk_prev = stk_cur
            conv23(stk_prev, nsb - 1)

            if not phase2:
                lg = wp.tile([2, 256], f32, name="lg")
                nc.vector.tensor_copy(lg[:], h2brep[0:2, 0:512].bitcast(f32))
                nc.sync.dma_start(logits_d[:], lg[:])
            if phase2:
                # ================= phase 2: locally-connected stack ===========
                # ---- lconv3: 13x13 -> 9x9, 5x5 taps ----
                for p in range(81):
                    i, j = p // 9, p % 9
                    lwt = lwp.tile([128, 640], bf16, tag="lw3p")
                    nc.sync.dma_start(lwt[:], lw3p_d[p])
                    lws = lwp.tile([64, 320], bf16, tag="lw3s")
                    nc.sync.dma_start(lws[:], lw3s_d[p])
                    ps = lps.tile([64, 256], f32, tag="lps")
                    for q in range(10):        # (u, v0/v0+1) pairs, K=128
                        u, v0 = q // 2, 2 * (q % 2)
                        col = ((i + u) * 13 + (j + v0)) * 256
                        nc.tensor.matmul(ps[:], lwt[:, 64 * q:64 * (q + 1)],
                                         h2brep[:, col:col + 256],
                                         start=(q == 0), stop=False)
                    for s in range(5):         # (u, v=4) singles, K=64
                        col = ((i + s) * 13 + (j + 4)) * 256
                        nc.tensor.matmul(ps[:], lws[:, 64 * s:64 * (s + 1)],
                                         h2brep[0:64, col:col + 256],
                                         start=False, stop=(s == 4))
                    nc.scalar.activation(h3rep[0:64, 256 * p:256 * (p + 1)], ps[:],
                                         Tanh, bias=lb3t[:, p:p + 1])
                    if p >= 1:
                        nc.scalar.activation(
                            h3rep[64:128, 256 * (p - 1):256 * p], ps[:],
                            Tanh, bias=lb3t[:, p:p + 1])

                # ---- lconv4: 9x9 -> 5x5, 5x5 taps ----
                for p in range(25):
                    i, j = p // 5, p % 5
                    lwt = lwp.tile([128, 640], bf16, tag="lw4p")
                    nc.sync.dma_start(lwt[:], lw4p_d[p])
                    lws = lwp.tile([64, 320], bf16, tag="lw4s")
                    nc.sync.dma_start(lws[:], lw4s_d[p])
                    ps = lps.tile([64, 256], f32, tag="lps")
                    for q in range(10):
                        u, v0 = q // 2, 2 * (q % 2)
                        col = ((i + u) * 9 + (j + v0)) * 256
                        nc.tensor.matmul(ps[:], lwt[:, 64 * q:64 * (q + 1)],
                                         h3rep[:, col:col + 256],
                                         start=(q == 0), stop=False)
                    for s in range(5):
                        col = ((i + s) * 9 + (j + 4)) * 256
                        nc.tensor.matmul(ps[:], lws[:, 64 * s:64 * (s + 1)],
                                         h3rep[0:64, col:col + 256],
                                         start=False, stop=(s == 4))
                    nc.scalar.activation(h4rep[0:64, 256 * p:256 * (p + 1)], ps[:],
                                         Tanh, bias=lb4t[:, p:p + 1])
                    if p >= 1:
                        nc.scalar.activation(
                            h4rep[64:128, 256 * (p - 1):256 * p], ps[:],
                            Tanh, bias=lb4t[:, p:p + 1])

                # ---- lconv5: 5x5 -> 3x3, 3x3 taps ----
                for p in range(9):
                    i, j = p // 3, p % 3
                    lwt = lwp.tile([128, 192], bf16, tag="lw5p")
                    nc.sync.dma_start(lwt[:], lw5p_d[p])
                    lws = lwp.tile([64, 192], bf16, tag="lw5s")
                    nc.sync.dma_start(lws[:], lw5s_d[p])
                    ps = lps.tile([64, 256], f32, tag="lps")
                    for q in range(3):         # (u, v=0/1) pairs
                        col = ((i + q) * 5 + (j + 0)) * 256
                        nc.tensor.matmul(ps[:], lwt[:, 64 * q:64 * (q + 1)],
                                         h4rep[:, col:col + 256],
                                         start=(q == 0), stop=False)
                    for s in range(3):         # (u, v=2) singles
                        col = ((i + s) * 5 + (j + 2)) * 256
                        nc.tensor.matmul(ps[:], lws[:, 64 * s:64 * (s + 1)],
                                         h4rep[0:64, col:col + 256],
                                         start=False, stop=(s == 2))
                    nc.scalar.activation(h5t[:, 256 * p:256 * (p + 1)], ps[:],
                                         Tanh, bias=lb5t[:, p:p + 1])

                # ---- head: logits[o, b] = sum_f hw[o, f] feat[f, b] (h5 part) ----
                psh = hps.tile([2, 256], f32)
                for yx in range(9):
                    nc.tensor.matmul(psh[:], hwch[:, 2 * yx:2 * yx + 2],
                                     h5t[:, 256 * yx:256 * (yx + 1)],
                                     start=(yx == 0), stop=(yx == 8))
                lg = wp.tile([2, 256], f32)
                nc.vector.tensor_copy(lg[:], psh[:])
                nc.sync.dma_start(logits_d[:], lg[:])

    nc.compile()
    return nc


def _prep_weights(w1, b1, w2a, b2a, w2b, b2b, lw3, lb3, lw4, lb4, lw5, lb5, hw):
    """Host-side reshape of weights into the on-chip matmul layouts."""
    out = {}
    w1di = np.zeros((60, 3, 128), np.float32)
    for g in range(4):
        for di in range(3):
            for dj in range(3):
                w1di[15 * g + 5 * di:15 * g + 5 * di + 5, dj,
                     32 * g:32 * g + 32] = w1[:, :, di, dj].T   # [5ci, 32co]
    out["w1di"] = w1di.reshape(60, 3 * 128).astype(BF)
    out["b1t"] = np.tile(b1, 4)[:, None].astype(np.float32)

    w2adi = np.zeros((96, 3, 64), np.float32)
    for di in range(3):
        for dj in range(3):
            w2adi[32 * di:32 * di + 32, dj, :] = w2a[:, :, di, dj].T  # [32ci, 64co]
    out["w2adi"] = w2adi.reshape(96, 3 * 64).astype(BF)
    out["b2at"] = np.tile(b2a, 2)[:, None].astype(np.float32)

    w2bbd = np.zeros((128, 9, 128), np.float32)
    for t, (di, dj) in enumerate(TAPS3):
        blk = w2b[:, :, di, dj].T         # [64ci, 64co]
        for g2 in range(2):
            w2bbd[64 * g2:64 * (g2 + 1), t, 64 * g2:64 * (g2 + 1)] = blk
    out["w2bbd"] = w2bbd.reshape(128, 9 * 128).astype(BF)
    out["b2bt"] = np.tile(b2b, 2)[:, None].astype(np.float32)

    def lc_pack(lw, Ho, Wo, kh, kw):
        npos = Ho * Wo
        npair = kh * (kw // 2)
        nsing = kh
        lp = np.zeros((npos, 128, npair * 64), np.float32)
        ls = np.zeros((npos, 64, nsing * 64), np.float32)
        for p in range(npos):
            i, j = p // Wo, p % Wo
            for q in range(npair):
                u, v0 = q // (kw // 2), 2 * (q % (kw // 2))
                lp[p, 0:64, 64 * q:64 * (q + 1)] = lw[i, j, :, :, u, v0].T
                lp[p, 64:128, 64 * q:64 * (q + 1)] = lw[i, j, :, :, u, v0 + 1].T
            for s in range(nsing):
                ls[p, 0:64, 64 * s:64 * (s + 1)] = lw[i, j, :, :, s, kw - 1].T
        return lp.astype(BF), ls.astype(BF)

    out["lw3p"], out["lw3s"] = lc_pack(lw3, 9, 9, 5, 5)
    out["lb3t"] = np.ascontiguousarray(
        lb3.transpose(2, 0, 1).reshape(64, 81)).astype(np.float32)
    out["lw4p"], out["lw4s"] = lc_pack(lw4, 5, 5, 5, 5)
    out["lb4t"] = np.ascontiguousarray(
        lb4.transpose(2, 0, 1).reshape(64, 25)).astype(np.float32)
    out["lw5p"], out["lw5s"] = lc_pack(lw5, 3, 3, 3, 3)
    out["lb5t"] = np.ascontiguousarray(
        lb5.transpose(2, 0, 1).reshape(64, 9)).astype(np.float32)

    # head: feature f = co*9 + yx; chunk yx -> [64co, 2]
    out["hwch"] = np.ascontiguousarray(
        hw[:, :576].reshape(2, 64, 9).transpose(1, 2, 0).reshape(64, 18)
    ).astype(BF)
    return out


def kernel(x, info, w1, b1, w2a, b2a, w2b, b2b, lw3, lb3, lw4, lb4, lw5, lb5,
           hw, hb, _trace=False):
    x = np.asarray(x, np.float32)
    if "nc" not in _CACHE:
        _CACHE["nc"] = _build_module()
    nc = _CACHE["nc"]

    wts = _prep_weights(np.asarray(w1, np.float32), np.asarray(b1, np.float32),
                        np.asarray(w2a, np.float32), np.asarray(b2a, np.float32),
                        np.asarray(w2b, np.float32), np.asarray(b2b, np.float32),
                        np.asarray(lw3, np.float32), np.asarray(lb3, np.float32),
                        np.asarray(lw4, np.float32), np.asarray(lb4, np.float32),
                        np.asarray(lw5, np.float32), np.asarray(lb5, np.float32),
                        np.asarray(hw, np.float32))

    # split rows by parity: plane 0 = even rows (30), plane 1 = odd rows (30)
    xr = x.reshape(B_FULL, 5, 30, 2, 60)
    xb = np.ascontiguousarray(
        np.stack([xr[:, :, :, 0, :], xr[:, :, :, 1, :]], axis=1)
        .reshape(B_FULL, 2, 5, 1800)).astype(BF)
    in_maps = []
    for c in range(N_CORES):
        m = dict(wts)
        m["x"] = xb[c * B_CORE:(c + 1) * B_CORE]
        in_maps.append(m)

    res = bass_utils.run_bass_kernel_spmd(
        nc, in_maps, core_ids=list(range(N_CORES)), trace=_trace)
    if _trace:
        _CACHE["last_results"] = res

    logits = np.concatenate(
        [res.results[c]["logits"].T for c in range(N_CORES)], axis=0)  # [2048, 2]

    # host-side tail: info contribution + bias + softmax (fp32)
    info = np.asarray(info, np.float32)
    hw = np.asarray(hw, np.float32)
    hb = np.asarray(hb, np.float32)
    logits = logits + info @ hw[:, 576:].T + hb[None, :]
    m = logits.max(axis=1, keepdims=True)
    e = np.exp(logits - m)
    return (e / e.sum(axis=1, keepdims=True)).astype(np.float32)


# revision 12
# speedup vs baseline: 1.6208x; 1.0022x over previous
"""Trainium2 Bass kernel for the DeepFace-style CNN (nn_DeepFace_10574209482846).

Sharding: pure data parallel — batch 2048 split as 256 images per core
across 8 cores; all weights replicated (host-preprocessed into matmul-
friendly layouts, cast to bf16).

Per-core layout: 256 images = 4 groups of 64; sub-batches of 8 images
(2 per group).  Dense convs contract the 3 row-taps (di) inside K via
physically row-shifted input stacks (built by DMA for conv1, by cheap
contiguous DVE copies for conv2a); the 3 column-taps (dj) are free-dim
offsets.  conv2b keeps the 2-group block-diagonal K=128 form (already
optimal).  Locally-connected layers use a batch-contiguous (y, x, b)
layout with a +1-column-shifted replica on partitions 64..127 (written
directly by dual scalar activations) so two kernel taps contract per
matmul (K=128).
"""

import numpy as np
import concourse.bass as bass
import concourse.bacc as bacc
import concourse.tile as tile
import concourse.mybir as mybir
from concourse import bass_utils

bf16 = mybir.dt.bfloat16
f32 = mybir.dt.float32
BF = mybir.dt.np(bf16)  # ml_dtypes.bfloat16

N_CORES = 8
B_FULL = 2048
B_CORE = 256          # images per core
SB = 8                # images per sub-batch (2 per group)
NSB = B_CORE // SB    # 32
BSB = SB // 4         # 2 images per group per sub-batch

TAPS3 = [(di, dj) for di in range(3) for dj in range(3)]

_CACHE = {}


def _build_module(nsb=NSB, phase2=True):
    nc = bacc.Bacc("TRN2", target_bir_lowering=False, debug=False,
                   enable_asserts=True, num_devices=N_CORES)

    # ---- DRAM I/O ----
    # x pre-split on host into row-parity planes: plane 0 = even rows
    # 0,2,..,58 (30), plane 1 = odd rows 1,3,..,59 (30).
    x_d = nc.dram_tensor("x", [B_CORE, 2, 5, 1800], bf16,
                         kind="ExternalInput").ap()
    w1di_d = nc.dram_tensor("w1di", [60, 3 * 128], bf16, kind="ExternalInput").ap()
    b1t_d = nc.dram_tensor("b1t", [128, 1], f32, kind="ExternalInput").ap()
    w2adi_d = nc.dram_tensor("w2adi", [96, 3 * 64], bf16, kind="ExternalInput").ap()
    b2at_d = nc.dram_tensor("b2at", [128, 1], f32, kind="ExternalInput").ap()
    w2bbd_d = nc.dram_tensor("w2bbd", [128, 9 * 128], bf16, kind="ExternalInput").ap()
    b2bt_d = nc.dram_tensor("b2bt", [128, 1], f32, kind="ExternalInput").ap()
    lw3p_d = nc.dram_tensor("lw3p", [81, 128, 640], bf16, kind="ExternalInput").ap()
    lw3s_d = nc.dram_tensor("lw3s", [81, 64, 320], bf16, kind="ExternalInput").ap()
    lb3_d = nc.dram_tensor("lb3t", [64, 81], f32, kind="ExternalInput").ap()
    lw4p_d = nc.dram_tensor("lw4p", [25, 128, 640], bf16, kind="ExternalInput").ap()
    lw4s_d = nc.dram_tensor("lw4s", [25, 64, 320], bf16, kind="ExternalInput").ap()
    lb4_d = nc.dram_tensor("lb4t", [64, 25], f32, kind="ExternalInput").ap()
    lw5p_d = nc.dram_tensor("lw5p", [9, 128, 192], bf16, kind="ExternalInput").ap()
    lw5s_d = nc.dram_tensor("lw5s", [9, 64, 192], bf16, kind="ExternalInput").ap()
    lb5_d = nc.dram_tensor("lb5t", [64, 9], f32, kind="ExternalInput").ap()
    hwch_d = nc.dram_tensor("hwch", [64, 18], bf16, kind="ExternalInput").ap()
    logits_d = nc.dram_tensor("logits", [2, B_CORE], f32, kind="ExternalOutput").ap()

    Tanh = mybir.ActivationFunctionType.Tanh

    with tile.TileContext(nc) as tc:
        with (
            tc.tile_pool(name="wp", bufs=1) as wp,
            tc.tile_pool(name="lwp", bufs=2) as lwp,
            tc.tile_pool(name="xp", bufs=2) as xp,
            tc.tile_pool(name="h1p", bufs=1) as h1p,
            tc.tile_pool(name="stkp", bufs=2) as stkp,
            tc.tile_pool(name="h2ap", bufs=1) as h2ap,
            tc.tile_pool(name="bigp", bufs=1) as bigp,
            tc.tile_pool(name="cps", bufs=5, space="PSUM") as cps,
            tc.tile_pool(name="lps", bufs=2, space="PSUM") as lps,
            tc.tile_pool(name="hps", bufs=1, space="PSUM") as hps,
        ):
            # ---- persistent weights ----
            w1di = wp.tile([60, 3 * 128], bf16)
            nc.sync.dma_start(w1di[:], w1di_d[:])
            b1t = wp.tile([128, 1], f32)
            nc.sync.dma_start(b1t[:], b1t_d[:])
            w2adi = wp.tile([96, 3 * 64], bf16)
            nc.sync.dma_start(w2adi[:], w2adi_d[:])
            b2at = wp.tile([128, 1], f32)
            nc.sync.dma_start(b2at[:], b2at_d[:])
            w2bbd = wp.tile([128, 9 * 128], bf16)
            nc.sync.dma_start(w2bbd[:], w2bbd_d[:])
            b2bt = wp.tile([128, 1], f32)
            nc.sync.dma_start(b2bt[:], b2bt_d[:])
            lb3t = wp.tile([64, 81], f32)
            nc.sync.dma_start(lb3t[:], lb3_d[:])
            lb4t = wp.tile([64, 25], f32)
            nc.sync.dma_start(lb4t[:], lb4_d[:])
            lb5t = wp.tile([64, 9], f32)
            nc.sync.dma_start(lb5t[:], lb5_d[:])
            hwch = wp.tile([64, 18], bf16)
            nc.sync.dma_start(hwch[:], hwch_d[:])

            # ---- persistent activations (batch-contiguous, (y, x, b)) ----
            h2brep = bigp.tile([128, 169 * 256], bf16)   # rows 0-63 h2b, 64-127 +1col
            h3rep = bigp.tile([128, 81 * 256], bf16)
            h4rep = bigp.tile([128, 25 * 256], bf16)
            h5t = bigp.tile([64, 9 * 256], bf16)

            h2bv = h2brep[:].rearrange("c (y x b) -> c b y x", y=13, x=13, b=256)

            # ================= phase 1: conv1 -> conv2a -> conv2b =============
            # software-pipelined: iter sb runs conv1(sb)+copies(sb), then
            # conv2a(sb-1)+conv2b(sb-1), so DVE copies hide under PE work.
            C1STRIPS = [(0, 8), (8, 8), (16, 8), (24, 5)]
            C2STRIPS = [(0, 9), (9, 9), (18, 9)]

            def load_x(sb):
                # row-shifted di-stack: partition (g, di, c) = 15g+5di+c,
                # free (b, y, x) with tile row y = input row 2y+di.
                xs = xp.tile([60, BSB * 29 * 60], bf16, tag="xs", name="xs")
                xsb = xs[:].rearrange("p (b m) -> p b m", b=BSB)
                for g in range(4):
                    b0 = 64 * g + BSB * sb
                    base = x_d[b0:b0 + BSB]          # [BSB, 2, 5, 1800]
                    for di, (pl, r0) in enumerate([(0, 0), (1, 0), (0, 1)]):
                        src = base[:, pl, :, 60 * r0:60 * r0 + 1740].rearrange(
                            "b c m -> c b m")
                        nc.sync.dma_start(
                            xsb[15 * g + 5 * di:15 * g + 5 * di + 5, :, :], src)
                return xs

            def conv1(xs):
                xsv = xs[:].rearrange("p (b y x) -> p b y x", b=BSB, y=29, x=60)
                h1 = h1p.tile([128, BSB * 841], bf16, tag="h1", name="h1")
                h1v = h1[:].rearrange("c (b h w) -> c b h w", b=BSB, h=29, w=29)
                for (y0, ny) in C1STRIPS:
                    ps = cps.tile([128, 486], f32, tag="cps", name="c1ps")
                    psw = ps[:, :BSB * ny * 29]
                    for dj in range(3):
                        rhs = xsv[:, :, y0:y0 + ny, dj:dj + 57:2]
                        nc.tensor.matmul(psw, w1di[:, 128 * dj:128 * (dj + 1)],
                                         rhs, start=(dj == 0), stop=(dj == 2))
                    nc.scalar.activation(h1v[:, :, y0:y0 + ny, :], psw, Tanh,
                                         bias=b1t[:])
                # di-stack for conv2a: per group [96 = 3di x 32c, (b y x)],
                # di shift = +29 elements (one y row), flat contiguous copy.
                stk = stkp.tile([96, 4 * 1682], bf16, tag="stk", name="stk")
                for g in range(4):
                    for di in range(3):
                        L = 1682 - 29 * di
                        nc.vector.tensor_copy(
                            stk[32 * di:32 * di + 32, 1682 * g:1682 * g + L],
                            h1[32 * g:32 * g + 32, 29 * di:29 * di + L])
                return stk

            def conv23(stk, sb):
                # ---- conv2a: per group, K=96 (3 di x 32 ci), dj by offset ----
                h2a_t = {}
                for r in range(2):
                    h2a_t[r] = h2ap.tile([128, BSB * 729], bf16, tag=f"h2a{r}",
                                         name=f"h2a{r}")
                for r in range(2):
                    h2av = h2a_t[r][:].rearrange("c (b h w) -> c b h w",
                                                 b=BSB, h=27, w=27)
                    for (y0, ny) in C2STRIPS:
                        ps = cps.tile([128, 486], f32, tag="cps", name="c2aps")
                        for half in range(2):
                            g = 2 * r + half
                            sv = stk[:, 1682 * g:1682 * (g + 1)].rearrange(
                                "p (b h w) -> p b h w", b=BSB, h=29, w=29)
                            psw = ps[64 * half:64 * half + 64, :BSB * ny * 27]
                            for dj in range(3):
                                rhs = sv[:, :, y0:y0 + ny, dj:dj + 27]
                                nc.tensor.matmul(
                                    psw, w2adi[:, 64 * dj:64 * (dj + 1)], rhs,
                                    start=(dj == 0), stop=(dj == 2))
                        nc.scalar.activation(
                            h2av[:, :, y0:y0 + ny, :],
                            ps[:, :BSB * ny * 27], Tanh, bias=b2at[:])

                # ---- conv2b: per pair, K=128 block-diag, stride 2 ----
                for r in range(2):
                    h2av = h2a_t[r][:].rearrange("c (b h w) -> c b h w",
                                                 b=BSB, h=27, w=27)
                    ps = cps.tile([128, 486], f32, tag="cps", name="c2bps")
                    psw = ps[:, :BSB * 169]
                    for t, (di, dj) in enumerate(TAPS3):
                        rhs = h2av[:, :, di: di + 25: 2, dj: dj + 25: 2]
                        nc.tensor.matmul(psw, w2bbd[:, 128 * t:128 * (t + 1)],
                                         rhs, start=(t == 0), stop=(t == 8))
                    # evacuate: psum rows (g-even 0:64 / g-odd 64:128) into
                    # h2brep[0:64, (y, x, b)] plus the +1-col replica rows.
                    psv = psw.rearrange("c (b y x) -> c b y x", b=BSB, y=13, x=13)
                    for g2 in range(2):
                        gb = 64 * (2 * r + g2) + BSB * sb
                        nc.scalar.activation(
                            h2bv[0:64, gb:gb + BSB, :, :],
                            psv[64 * g2:64 * (g2 + 1), :, :, :], Tanh,
                            bias=b2bt[64 * g2:64 * (g2 + 1)])
                        # +1-col replica rows: pure copy on the (idle) DVE
                        rep = h2brep[64:128, 0:168 * 256].rearrange(
                            "c (yx b) -> c b yx", b=256)[:, gb:gb + BSB, :]
                        srcr = h2brep[0:64, 256:256 + 168 * 256].rearrange(
                            "c (yx b) -> c b yx", b=256)[:, gb:gb + BSB, :]
                        nc.vector.tensor_copy(rep, srcr)

            xs_cur = load_x(0)
            stk_prev = None
            for sb in range(nsb):
                stk_cur = conv1(xs_cur)
                if sb + 1 < nsb:
                    xs_cur = load_x(sb + 1)
                if stk_prev is not None:
                    conv23(stk_prev, sb - 1)
                stk_prev = stk_cur
            conv23(stk_prev, nsb - 1)

            if not phase2:
                lg = wp.tile([2, 256], f32, name="lg")
                nc.vector.tensor_copy(lg[:], h2brep[0:2, 0:512].bitcast(f32))
                nc.sync.dma_start(logits_d[:], lg[:])
            if phase2:
                # ================= phase 2: locally-connected stack ===========
                # ---- lconv3: 13x13 -> 9x9, 5x5 taps ----
                for p in range(81):
                    i, j = p // 9, p % 9
                    lwt = lwp.tile([128, 640], bf16, tag="lw3p")
                    nc.sync.dma_start(lwt[:], lw3p_d[p])
                    lws = lwp.tile([64, 320], bf16, tag="lw3s")
                    nc.sync.dma_start(lws[:], lw3s_d[p])
                    ps = lps.tile([64, 256], f32, tag="lps")
                    for q in range(10):        # (u, v0/v0+1) pairs, K=128
                        u, v0 = q // 2, 2 * (q % 2)
                        col = ((i + u) * 13 + (j + v0)) * 256
                        nc.tensor.matmul(ps[:], lwt[:, 64 * q:64 * (q + 1)],
                                         h2brep[:, col:col + 256],
                                         start=(q == 0), stop=False)
                    for s in range(5):         # (u, v=4) singles, K=64
                        col = ((i + s) * 13 + (j + 4)) * 256
                        nc.tensor.matmul(ps[:], lws[:, 64 * s:64 * (s + 1)],
                                         h2brep[0:64, col:col + 256],
                                         start=False, stop=(s == 4))
                    nc.scalar.activation(h3rep[0:64, 256 * p:256 * (p + 1)], ps[:],
                                         Tanh, bias=lb3t[:, p:p + 1])
                    if p >= 1:
                        nc.vector.tensor_copy(
                            h3rep[64:128, 256 * (p - 1):256 * p],
                            h3rep[0:64, 256 * p:256 * (p + 1)])

                # ---- lconv4: 9x9 -> 5x5, 5x5 taps ----
                for p in range(25):
                    i, j = p // 5, p % 5
                    lwt = lwp.tile([128, 640], bf16, tag="lw4p")
                    nc.sync.dma_start(lwt[:], lw4p_d[p])
                    lws = lwp.tile([64, 320], bf16, tag="lw4s")
                    nc.sync.dma_start(lws[:], lw4s_d[p])
                    ps = lps.tile([64, 256], f32, tag="lps")
                    for q in range(10):
                        u, v0 = q // 2, 2 * (q % 2)
                        col = ((i + u) * 9 + (j + v0)) * 256
                        nc.tensor.matmul(ps[:], lwt[:, 64 * q:64 * (q + 1)],
                                         h3rep[:, col:col + 256],
                                         start=(q == 0), stop=False)
                    for s in range(5):
                        col = ((i + s) * 9 + (j + 4)) * 256
                        nc.tensor.matmul(ps[:], lws[:, 64 * s:64 * (s + 1)],
                                         h3rep[0:64, col:col + 256],
                                         start=False, stop=(s == 4))
                    nc.scalar.activation(h4rep[0:64, 256 * p:256 * (p + 1)], ps[:],
                                         Tanh, bias=lb4t[:, p:p + 1])
                    if p >= 1:
                        nc.vector.tensor_copy(
                            h4rep[64:128, 256 * (p - 1):256 * p],
                            h4rep[0:64, 256 * p:256 * (p + 1)])

                # ---- lconv5: 5x5 -> 3x3, 3x3 taps ----
                for p in range(9):
                    i, j = p // 3, p % 3
                    lwt = lwp.tile([128, 192], bf16, tag="lw5p")
                    nc.sync.dma_start(lwt[:], lw5p_d[p])
                    lws = lwp.tile([64, 192], bf16, tag="lw5s")
                    nc.sync.dma_start(lws[:], lw5s_d[p])
                    ps = lps.tile([64, 256], f32, tag="lps")
                    for q in range(3):         # (u, v=0/1) pairs
                        col = ((i + q) * 5 + (j + 0)) * 256
                        nc.tensor.matmul(ps[:], lwt[:, 64 * q:64 * (q + 1)],
                                         h4rep[:, col:col + 256],
                                         start=(q == 0), stop=False)
                    for s in range(3):         # (u, v=2) singles
                        col = ((i + s) * 5 + (j + 2)) * 256
                        nc.tensor.matmul(ps[:], lws[:, 64 * s:64 * (s + 1)],
                                         h4rep[0:64, col:col + 256],
                                         start=False, stop=(s == 2))
                    nc.scalar.activation(h5t[:, 256 * p:256 * (p + 1)], ps[:],
                                         Tanh, bias=lb5t[:, p:p + 1])

                # ---- head: logits[o, b] = sum_f hw[o, f] feat[f, b] (h5 part) ----
                psh = hps.tile([2, 256], f32)
                for yx in range(9):
                    nc.tensor.matmul(psh[:], hwch[:, 2 * yx:2 * yx + 2],
                                     h5t[:, 256 * yx:256 * (yx + 1)],
                                     start=(yx == 0), stop=(yx == 8))
                lg = wp.tile([2, 256], f32)
                nc.vector.tensor_copy(lg[:], psh[:])
                nc.sync.dma_start(logits_d[:], lg[:])

    nc.compile()
    return nc


def _prep_weights(w1, b1, w2a, b2a, w2b, b2b, lw3, lb3, lw4, lb4, lw5, lb5, hw):
    """Host-side reshape of weights into the on-chip matmul layouts."""
    out = {}
    w1di = np.zeros((60, 3, 128), np.float32)
    for g in range(4):
        for di in range(3):
            for dj in range(3):
                w1di[15 * g + 5 * di:15 * g + 5 * di + 5, dj,
                     32 * g:32 * g + 32] = w1[:, :, di, dj].T   # [5ci, 32co]
    out["w1di"] = w1di.reshape(60, 3 * 128).astype(BF)
    out["b1t"] = np.tile(b1, 4)[:, None].astype(np.float32)

    w2adi = np.zeros((96, 3, 64), np.float32)
    for di in range(3):
        for dj in range(3):
            w2adi[32 * di:32 * di + 32, dj, :] = w2a[:, :, di, dj].T  # [32ci, 64co]
    out["w2adi"] = w2adi.reshape(96, 3 * 64).astype(BF)
    out["b2at"] = np.tile(b2a, 2)[:, None].astype(np.float32)

    w2bbd = np.zeros((128, 9, 128), np.float32)
    for t, (di, dj) in enumerate(TAPS3):
        blk = w2b[:, :, di, dj].T         # [64ci, 64co]
        for g2 in range(2):
            w2bbd[64 * g2:64 * (g2 + 1), t, 64 * g2:64 * (g2 + 1)] = blk
    out["w2bbd"] = w2bbd.reshape(128, 9 * 128).astype(BF)
    out["b2bt"] = np.tile(b2b, 2)[:, None].astype(np.float32)

    def lc_pack(lw, Ho, Wo, kh, kw):
        npos = Ho * Wo
        npair = kh * (kw // 2)
        nsing = kh
        lp = np.zeros((npos, 128, npair * 64), np.float32)
        ls = np.zeros((npos, 64, nsing * 64), np.float32)
        for p in range(npos):
            i, j = p // Wo, p % Wo
            for q in range(npair):
                u, v0 = q // (kw // 2), 2 * (q % (kw // 2))
                lp[p, 0:64, 64 * q:64 * (q + 1)] = lw[i, j, :, :, u, v0].T
                lp[p, 64:128, 64 * q:64 * (q + 1)] = lw[i, j, :, :, u, v0 + 1].T
            for s in range(nsing):
                ls[p, 0:64, 64 * s:64 * (s + 1)] = lw[i, j, :, :, s, kw - 1].T
        return lp.astype(BF), ls.astype(BF)

    out["lw3p"], out["lw3s"] = lc_pack(lw3, 9, 9, 5, 5)
    out["lb3t"] = np.ascontiguousarray(
        lb3.transpose(2, 0, 1).reshape(64, 81)).astype(np.float32)
    out["lw4p"], out["lw4s"] = lc_pack(lw4, 5, 5, 5, 5)
    out["lb4t"] = np.ascontiguousarray(
        lb4.transpose(2, 0, 1).reshape(64, 25)).astype(np.float32)
    out["lw5p"], out["lw5s"] = lc_pack(lw5, 3, 3, 3, 3)
    out["lb5t"] = np.ascontiguousarray(
        lb5.transpose(2, 0, 1).reshape(64, 9)).astype(np.float32)

    # head: feature f = co*9 + yx; chunk yx -> [64co, 2]
    out["hwch"] = np.ascontiguousarray(
        hw[:, :576].reshape(2, 64, 9).transpose(1, 2, 0).reshape(64, 18)
    ).astype(BF)
    return out


def kernel(x, info, w1, b1, w2a, b2a, w2b, b2b, lw3, lb3, lw4, lb4, lw5, lb5,
           hw, hb, _trace=False):
    x = np.asarray(x, np.float32)
    if "nc" not in _CACHE:
        _CACHE["nc"] = _build_module()
    nc = _CACHE["nc"]

    wts = _prep_weights(np.asarray(w1, np.float32), np.asarray(b1, np.float32),
                        np.asarray(w2a, np.float32), np.asarray(b2a, np.float32),
                        np.asarray(w2b, np.float32), np.asarray(b2b, np.float32),
                        np.asarray(lw3, np.float32), np.asarray(lb3, np.float32),
                        np.asarray(lw4, np.float32), np.asarray(lb4, np.float32),
                        np.asarray(lw5, np.float32), np.asarray(lb5, np.float32),
                        np.asarray(hw, np.float32))

    # split rows by parity: plane 0 = even rows (30), plane 1 = odd rows (30)
    xr = x.reshape(B_FULL, 5, 30, 2, 60)
    xb = np.ascontiguousarray(
        np.stack([xr[:, :, :, 0, :], xr[:, :, :, 1, :]], axis=1)
        .reshape(B_FULL, 2, 5, 1800)).astype(BF)
    in_maps = []
    for c in range(N_CORES):
        m = dict(wts)
        m["x"] = xb[c * B_CORE:(c + 1) * B_CORE]
        in_maps.append(m)

    res = bass_utils.run_bass_kernel_spmd(
        nc, in_maps, core_ids=list(range(N_CORES)), trace=_trace)
    if _trace:
        _CACHE["last_results"] = res

    logits = np.concatenate(
        [res.results[c]["logits"].T for c in range(N_CORES)], axis=0)  # [2048, 2]

    # host-side tail: info contribution + bias + softmax (fp32)
    info = np.asarray(info, np.float32)
    hw = np.asarray(hw, np.float32)
    hb = np.asarray(hb, np.float32)
    logits = logits + info @ hw[:, 576:].T + hb[None, :]
    m = logits.max(axis=1, keepdims=True)
    e = np.exp(logits - m)
    return (e / e.sum(axis=1, keepdims=True)).astype(np.float32)
